# revision 62
# baseline (speedup 1.0000x reference)
"""Trainium2 Bass kernel for nn_MultiHeadAttention_76295799046818.

MHA: B=2, S=2048, D=1024, H=16 heads (d_k=64), causal, fp32 reference.
Sharded over 8 NeuronCores: data-parallel over batch (2) x tensor-parallel
over heads (4 heads/core).  Wq/Wk/Wv column-parallel; Wo row-parallel with
the 4 partial outputs per batch summed on the host (cheaper than an
on-device all-reduce in this runtime).

Single fused pipeline per core (bf16 matmuls, fp32 PSUM):
  - Projections are chunked by 512 rows of S and interleaved with attention:
    chunk sc feeds attention q-chunk qc=sc, so QK/exp start ~10us in while
    later chunks still stream from HBM.
  - Per (head-pair hp, q-chunk qc) sweep: QK + exp per k-tile, with the
    sweep's AV matmuls DEFERRED one sweep and interleaved into the next
    sweep's k-tile loop (keeps the PE stream dense and gives the slow
    gpsimd semaphore hop a whole sweep of slack).  The pair's two heads
    sit at SBUF partitions 0-63/64-127 so their K=64 QK matmuls run on
    disjoint PE row groups concurrently.  exp is sliced to live columns;
    the causal staircase inside diagonal tiles is zeroed by gpsimd
    affine_select on the bf16 exp tile (no PE mask matmuls, no mask DMA).
  - AV accumulates [V|ones] / [ones|0|V] weights so softmax denominators
    land at PSUM rows 64 (even head) / 0 (odd head); C^T raw is copied out
    bf16 immediately (frees PSUM); the raw denominators are broadcast
    across partitions by a one-hot PE matmul, reciprocal'd with DVE
    reciprocal_approx_fast (only works on partition-base-0 SBUF APs), and
    a DVE multiply normalizes - trailing the sweep by one more slot.
  - Wo partials per q-chunk trail two slots behind, interleaved as PE
    filler units (with next-chunk projections) into the Scalar-paced
    hp=0 sweeps; output bias via Scalar ACT, bf16 DMA out.
"""

import numpy as np
import ml_dtypes

import concourse.bass as bass
import concourse.mybir as mybir
import concourse.tile as tile
from concourse import bacc
from concourse.bass_utils import run_bass_kernel_spmd

BF16 = ml_dtypes.bfloat16

B, S, D, H, DK = 2, 2048, 1024, 16, 64
N_CORES = 8
TP = 4  # head-parallel degree (per batch)
HPC = H // TP  # heads per core = 4
O = HPC * DK  # output channels per core = 256
QT_BLK = 512
N_QC = S // QT_BLK  # 4
KC = D // 128  # 8 contraction chunks for projections

_CACHE = {}


def _build():
    nc = bacc.Bacc("TRN2", target_bir_lowering=False, debug=False,
                   num_devices=N_CORES)
    dt = mybir.dt
    f32, bf16, f32r = dt.float32, dt.bfloat16, dt.float32r

    def din(name, shape, dtype=bf16):
        return nc.dram_tensor(name, shape, dtype, kind="ExternalInput").ap()

    xqt_d = din("xqt", [N_QC, 128, KC, QT_BLK])
    xkt_d = din("xkt", [N_QC, 128, KC, QT_BLK])
    xvt_d = din("xvt", [N_QC, 128, KC, QT_BLK])
    wqt_d = din("wqt", [128, KC, O])
    wkt_d = din("wkt", [128, KC, O])
    wvt_d = din("wvt", [128, KC, O])
    wot_d = din("wot", [128, 2, D])
    bq_d = din("bqc", [128, 2], f32)
    bk_d = din("bkc", [128, 2], f32)
    bvb_d = din("bvb", [128, O], f32)
    bo_d = din("boc", [128, 8], f32)
    bsel_d = din("bsel", [65, 128], f32r)
    out_d = nc.dram_tensor("out", [8, 128, S], bf16,
                           kind="ExternalOutput").ap()

    EXPF = mybir.ActivationFunctionType.Exp
    IDF = mybir.ActivationFunctionType.Identity

    with tile.TileContext(nc) as tc:
        with (
            tc.tile_pool(name="const", bufs=1) as cpool,
            tc.tile_pool(name="xin", bufs=2) as xpool,
            tc.tile_pool(name="expp", bufs=34) as epool,
            tc.tile_pool(name="crp", bufs=3) as crpool,
            tc.tile_pool(name="ctp", bufs=2) as ctpool,
            tc.tile_pool(name="outp", bufs=4) as opool,
        ):
            # hot-path weights + first x chunks, in priority order: the K
            # projection of chunk 0 gates everything, so its DMAs go first
            # smallest possible first slices so the K chain's kc=0 matmul
            # fires as early as the DMA pipe allows
            wk_sb = cpool.tile([128, KC, O], bf16, name="wk_sb")
            nc.sync.dma_start(wk_sb[:, 0:2, :], wkt_d[:, 0:2, :])
            nc.sync.dma_start(wk_sb[:, 2:8, :], wkt_d[:, 2:8, :])
            xk0 = xpool.tile([128, KC, QT_BLK], bf16, name="xk", tag="xk")
            nc.sync.dma_start(xk0[:, 0:1, :], xkt_d[0][:, 0:1, :])
            nc.sync.dma_start(xk0[:, 1:2, :], xkt_d[0][:, 1:2, :])
            nc.sync.dma_start(xk0[:, 2:4, :], xkt_d[0][:, 2:4, :])
            nc.sync.dma_start(xk0[:, 4:6, :], xkt_d[0][:, 4:6, :])
            nc.sync.dma_start(xk0[:, 6:8, :], xkt_d[0][:, 6:8, :])
            wq_sb = cpool.tile([128, KC, O], bf16, name="wq_sb")
            nc.sync.dma_start(wq_sb[:, 0:4, :], wqt_d[:, 0:4, :])
            nc.sync.dma_start(wq_sb[:, 4:8, :], wqt_d[:, 4:8, :])
            xq0 = xpool.tile([128, KC, QT_BLK], bf16, name="xq", tag="xq")
            nc.sync.dma_start(xq0[:, 0:4, :], xqt_d[0][:, 0:4, :])
            nc.sync.dma_start(xq0[:, 4:8, :], xqt_d[0][:, 4:8, :])
            wv_sb = cpool.tile([128, KC, O], bf16, name="wv_sb")
            nc.sync.dma_start(wv_sb[:, 0:4, :], wvt_d[:, 0:4, :])
            nc.sync.dma_start(wv_sb[:, 4:8, :], wvt_d[:, 4:8, :])
            xv0 = xpool.tile([128, KC, QT_BLK], bf16, name="xv", tag="xv")
            nc.sync.dma_start(xv0[:, 0:4, :], xvt_d[0][:, 0:4, :])
            nc.sync.dma_start(xv0[:, 4:8, :], xvt_d[0][:, 4:8, :])
            bq_sb = cpool.tile([128, 2], f32, name="bq_sb")
            nc.sync.dma_start(bq_sb[:], bq_d[:])
            bk_sb = cpool.tile([128, 2], f32, name="bk_sb")
            nc.sync.dma_start(bk_sb[:], bk_d[:])
            bvb_sb = cpool.tile([128, O], f32, name="bvb_sb")
            nc.sync.dma_start(bvb_sb[:], bvb_d[:])

            qt_sb = cpool.tile([128, 2, S], bf16, name="qt_sb")
            kt_sb = cpool.tile([128, 2, S], bf16, name="kt_sb")
            # AV weights: per k-tile/pair, even head [V|ones] (den @ row 64),
            # odd head [ones|0|V] (den @ row 0, C^T @ rows 64-127)
            vaug_e = cpool.tile([128, 16, 2, 66], bf16, name="vaug_e")
            nc.vector.memset(vaug_e[:], 1.0)
            vaug_o = cpool.tile([128, 16, 2, 128], bf16, name="vaug_o")
            nc.vector.memset(vaug_o[:], 0.0)
            nc.vector.memset(vaug_o[:, :, :, 0:1], 1.0)
            # raw-denominator staging rows 0 (odd head) / 64 (even head);
            # rows 1-63 stay 1.0 (multiplied by bsel zeros in the bcast).
            # memset can't emit f32r, so memset f32 and cast-copy once.
            onesf = cpool.tile([65, QT_BLK], f32, name="onesf")
            nc.vector.memset(onesf[:], 1.0)
            dsb = cpool.tile([65, 2, QT_BLK], f32r, name="dsb")
            nc.vector.tensor_copy(dsb[:, 0, :], onesf[:])
            nc.vector.tensor_copy(dsb[:, 1, :], onesf[:])

            xq_t, xk_t, xv_t = {}, {}, {}

            def dma_chunk(sc, eng=None):
                eng = eng or nc.sync
                xk = xpool.tile([128, KC, QT_BLK], bf16, name="xk", tag="xk")
                eng.dma_start(xk[:, 0:4, :], xkt_d[sc][:, 0:4, :])
                eng.dma_start(xk[:, 4:8, :], xkt_d[sc][:, 4:8, :])
                xq = xpool.tile([128, KC, QT_BLK], bf16, name="xq", tag="xq")
                eng.dma_start(xq[:, 0:4, :], xqt_d[sc][:, 0:4, :])
                eng.dma_start(xq[:, 4:8, :], xqt_d[sc][:, 4:8, :])
                xv = xpool.tile([128, KC, QT_BLK], bf16, name="xv", tag="xv")
                eng.dma_start(xv[:, 0:4, :], xvt_d[sc][:, 0:4, :])
                eng.dma_start(xv[:, 4:8, :], xvt_d[sc][:, 4:8, :])
                xk_t[sc], xq_t[sc], xv_t[sc] = xk, xq, xv

            with tc.tile_pool(name="ps", bufs=2, space="PSUM") as ps:

                def proj_units(sc):
                    """Projection for chunk sc as a list of small emission
                    units (a few matmuls each) so they can be interleaved
                    into attention sweeps as PE filler."""
                    ssl = bass.ds(sc * QT_BLK, QT_BLK)
                    xk, xq, xv = xk_t[sc], xq_t[sc], xv_t[sc]
                    units = []

                    def chain(w_sb, x, b_sb, dst_tile, dst_hp, ot):
                        box = {}

                        def u1():
                            box["p"] = ps.tile([128, QT_BLK], f32, name="pp",
                                               tag="big", bufs=2)
                            for kc in range(4):
                                nc.tensor.matmul(
                                    box["p"][:],
                                    w_sb[:, kc, bass.ds(ot * 128, 128)],
                                    x[:, kc, :], start=(kc == 0), stop=False,
                                    skip_group_check=True)

                        def u2():
                            for kc in range(4, KC):
                                nc.tensor.matmul(
                                    box["p"][:],
                                    w_sb[:, kc, bass.ds(ot * 128, 128)],
                                    x[:, kc, :], start=False,
                                    stop=(kc == KC - 1),
                                    skip_group_check=True)
                            nc.scalar.activation(dst_tile[:, dst_hp, ssl],
                                                 box["p"][:], IDF,
                                                 bias=b_sb[:, ot:ot + 1])
                        units.extend([u1, u2])

                    for ot in range(2):
                        chain(wk_sb, xk, bk_sb, kt_sb, ot, ot)
                    for ot in range(2):
                        chain(wq_sb, xq, bq_sb, qt_sb, ot, ot)

                    def vchain(mtp):
                        box = {}

                        def u1():
                            box["p"] = ps.tile([128, QT_BLK], f32, name="pp",
                                               tag="big", bufs=2)
                            for kc in range(KC):
                                nc.tensor.matmul(
                                    box["p"][:, 0:O],
                                    xv[:, kc, bass.ds(2 * mtp * 128, 128)],
                                    wv_sb[:, kc, :], start=(kc == 0),
                                    stop=(kc == KC - 1),
                                    skip_group_check=True)

                        def u2():
                            for kc in range(KC):
                                nc.tensor.matmul(
                                    box["p"][:, O:2 * O],
                                    xv[:, kc,
                                       bass.ds((2 * mtp + 1) * 128, 128)],
                                    wv_sb[:, kc, :], start=(kc == 0),
                                    stop=(kc == KC - 1),
                                    skip_group_check=True)
                            pv = box["p"]
                            pvr = pv[:].rearrange(
                                "p (mt hp two d) -> p mt hp two d",
                                mt=2, hp=2, two=2)
                            bvr = bvb_sb[:].rearrange(
                                "p (hp two d) -> p hp two d", hp=2, two=2)
                            for mt2 in range(2):
                                kt = sc * 4 + 2 * mtp + mt2
                                nc.vector.tensor_tensor(
                                    vaug_e[:, kt, :, 0:64],
                                    pvr[:, mt2, :, 0, :], bvr[:, :, 0, :],
                                    mybir.AluOpType.add)
                                nc.vector.tensor_tensor(
                                    vaug_o[:, kt, :, 64:128],
                                    pvr[:, mt2, :, 1, :], bvr[:, :, 1, :],
                                    mybir.AluOpType.add)
                        units.extend([u1, u2])

                    for mtp in range(2):
                        vchain(mtp)
                    return units

                def proj(sc, mid=None):
                    units = proj_units(sc)
                    for i, u in enumerate(units):
                        if i == 4 and mid is not None:
                            mid()
                        u()

                xk_t[0], xq_t[0], xv_t[0] = xk0, xq0, xv0
                # chunk-1 + tail constants dispatch from the Scalar DGE,
                # queued behind proj(0)'s K-bias ACTs, so the in-flight DMA
                # set stays small while xq0/xv0 stream (packets of all
                # in-flight DMAs share the queues round-robin)
                consts = {}

                def _lead_mid():
                    dma_chunk(1)
                    bsel_sb = cpool.tile([65, 128], f32r, name="bsel_sb")
                    nc.sync.dma_start(bsel_sb[:], bsel_d[:])
                    wo_sb = cpool.tile([128, 2, D], bf16, name="wo_sb")
                    nc.sync.dma_start(wo_sb[:, 0, :], wot_d[:, 0, :])
                    nc.sync.dma_start(wo_sb[:, 1, :], wot_d[:, 1, :])
                    bo_sb = cpool.tile([128, 8], f32, name="bo_sb")
                    nc.sync.dma_start(bo_sb[:], bo_d[:])
                    consts.update(bsel_sb=bsel_sb, wo_sb=wo_sb, bo_sb=bo_sb)

                proj(0, mid=_lead_mid)
                bsel_sb = consts["bsel_sb"]
                wo_sb = consts["wo_sb"]
                bo_sb = consts["bo_sb"]

                ct_t = {}
                prev = None

                def bcast_norm(state):
                    # broadcast raw dens across partitions (PE), reciprocal
                    # in SBUF at partition base 0 (recip_approx_fast only
                    # works there), then normalize ctraw -> ct
                    qc, hp, ctraw, pp = state
                    pdup = ps.tile([128, QT_BLK], f32, name="pdup",
                                   tag="big", bufs=2)
                    nc.tensor.matmul(pdup[:], bsel_sb[:], dsb[0:65, pp, :],
                                     start=True, stop=True)
                    pbcs = crpool.tile([128, QT_BLK], f32, name="pbcs",
                                       tag="pbcs")
                    nc.vector.tensor_copy(pbcs[:], pdup[:])
                    pbcr = crpool.tile([128, QT_BLK], f32, name="pbcr",
                                       tag="pbcr")
                    nc.vector.reciprocal_approx_fast(pbcr[:], pbcs[:])
                    if hp == 0:
                        ct = ctpool.tile([128, 2, QT_BLK], bf16, name="ct",
                                         tag="ct")
                        ct_t[qc] = ct
                    ct = ct_t[qc]
                    nc.vector.tensor_tensor(ct[:, hp, :], ctraw[:, :],
                                            pbcr[:, :], mybir.AluOpType.mult)

                def wo_units(qc, jts=range(8), bias_split=False,
                             dma_split=1):
                    ct = ct_t[qc]
                    units = []

                    def mk(jt):
                        def u():
                            pwo = ps.tile([128, QT_BLK], f32, name="pwo",
                                          tag="big", bufs=2)
                            for kc in range(2):
                                nc.tensor.matmul(
                                    pwo[:],
                                    wo_sb[:, kc, bass.ds(jt * 128, 128)],
                                    ct[:, kc, :], start=(kc == 0),
                                    stop=(kc == 1), skip_group_check=True)
                            if bias_split and jt % 2:
                                # DVE path uses its own pool tag: one writer
                                # engine per tag (mixed writers on one tag
                                # proved race-prone on this runtime)
                                osb = opool.tile([128, QT_BLK], bf16,
                                                 name="osbv", tag="osbv",
                                                 bufs=2)
                                nc.vector.tensor_scalar(
                                    osb[:], pwo[:], bo_sb[:, jt:jt + 1],
                                    None, mybir.AluOpType.add)
                            else:
                                osb = opool.tile([128, QT_BLK], bf16,
                                                 name="osb", tag="osb")
                                nc.scalar.activation(osb[:], pwo[:], IDF,
                                                     bias=bo_sb[:, jt:jt + 1])
                            # a single 128KB dma rides ~one queue (~27GB/s);
                            # split the drain-phase stores across queues so
                            # the end-of-kernel barrier isn't stuck on them
                            w = QT_BLK // dma_split
                            for sp in range(dma_split):
                                qs = bass.ds(qc * QT_BLK + sp * w, w)
                                nc.sync.dma_start(out_d[jt][:, qs],
                                                  osb[:, bass.ds(sp * w, w)])
                        return u

                    for jt in jts:
                        units.append(mk(jt))
                    return units

                def wo(qc, jts=range(8), bias_split=False, dma_split=1):
                    for u in wo_units(qc, jts, bias_split, dma_split):
                        u()

                # Sweep-level software pipeline: sweep N emits only QK+exp
                # (+affine); its AV matmuls are interleaved into sweep N+1's
                # k-tile loop.  This keeps the PE stream dense (long streaks
                # ramp the PE p-state) and gives the slow gpsimd affine hop a
                # whole sweep of slack before its AV consumer.
                def emit_av(s, idx):
                    ki, et, lo = s["ets"][idx]
                    alo = 0 if idx == 0 else lo
                    last = s["n_ki"] - 1
                    nc.tensor.matmul(
                        s["pave"][:, alo:QT_BLK],
                        vaug_e[:, ki, s["hp"], 0:65],
                        et[:, 0, alo:QT_BLK],
                        start=(idx == 0), stop=(idx == last),
                        skip_group_check=True)
                    nc.tensor.matmul(
                        s["pavo"][:, alo:QT_BLK],
                        vaug_o[:, ki, s["hp"], :],
                        et[:, 1, alo:QT_BLK],
                        start=(idx == 0), stop=(idx == last),
                        skip_group_check=True)

                def dens(s):
                    # raw denominators first (they gate the pdup broadcast),
                    # then raw C^T out of PSUM (frees pav)
                    pp = s["hp"]
                    nc.vector.tensor_copy(dsb[64:65, pp, :],
                                          s["pave"][64:65, :])
                    nc.vector.tensor_copy(dsb[0:1, pp, :],
                                          s["pavo"][0:1, :])
                    ctraw = crpool.tile([128, QT_BLK], bf16,
                                        name="ctraw", tag="ctraw")
                    nc.vector.tensor_copy(ctraw[0:64, :], s["pave"][0:64, :])
                    nc.vector.tensor_copy(ctraw[64:128, :],
                                          s["pavo"][64:128, :])
                    return (s["qc"], s["hp"], ctraw, pp)

                prev_sw = None   # sweep whose AVs run during current sweep
                pend_norm = None  # dens() output awaiting bcast_norm
                wo_ready = []    # qc values whose ct is fully normalized

                for qc in range(N_QC):
                    for hp in range(2):
                        n_ki = 4 * qc + 4
                        if prev_sw is not None:
                            prev_sw["pave"] = ps.tile([65, QT_BLK], f32,
                                                      name="pav_e",
                                                      tag="pave", bufs=1)
                            prev_sw["pavo"] = ps.tile([128, QT_BLK], f32,
                                                      name="pav_o",
                                                      tag="pavo", bufs=1)
                        n_av = len(prev_sw["ets"]) if prev_sw else 0
                        av_done = 0
                        # PE filler for Scalar-paced hp=0 sweeps: the next
                        # chunk's projections and the pending Wo, emitted in
                        # small units between k-tiles (their input data
                        # landed sweeps ago)
                        fill = []
                        if hp == 0 and qc >= 2:
                            if qc < 3:
                                fill += proj_units(qc + 1)
                            if wo_ready:
                                # bias_split: half the bias ops go to DVE so
                                # they don't sit in the Scalar stream ahead
                                # of this sweep's exps (Scalar paces hp=0
                                # sweeps)
                                fill += wo_units(wo_ready.pop(0),
                                                 bias_split=True)
                        f_done = 0
                        # last sweep: front-load the previous sweep's AVs,
                        # free its pav mid-sweep, then run our own AVs
                        # inline so the flush only drains a couple of tiles
                        # (tried: inline last-sweep AVs; regressed 27us -
                        # front-loaded AVs starve the st/exp pipeline)
                        special = False
                        sp_state = None
                        sp_norm = None
                        sp_done = 0
                        ets = []
                        for ki in range(n_ki):
                            lo = max(0, 128 * ki - QT_BLK * qc)
                            st = ps.tile([128, 2, QT_BLK], f32, name="st",
                                         tag="st", bufs=2)
                            for side in range(2):
                                po = bass.ds(side * 64, 64)
                                nc.tensor.matmul(
                                    st[:, side, lo:QT_BLK],
                                    kt_sb[po, hp, bass.ds(ki * 128, 128)],
                                    qt_sb[po, hp,
                                          bass.ds(qc * QT_BLK + lo,
                                                  QT_BLK - lo)],
                                    start=True, stop=True)
                            et = epool.tile([128, 2, QT_BLK], bf16,
                                            name="et", tag="et")
                            nc.scalar.activation(et[:, :, lo:QT_BLK],
                                                 st[:, :, lo:QT_BLK], EXPF,
                                                 scale=0.125)
                            if ki >= 4 * qc:
                                # zero the causal staircase (cols lo..lo+127)
                                nc.gpsimd.affine_select(
                                    out=et[:, :, bass.ds(lo, 128)],
                                    in_=et[:, :, bass.ds(lo, 128)],
                                    compare_op=mybir.AluOpType.is_ge,
                                    fill=0.0, base=0,
                                    pattern=[[0, 2], [1, 128]],
                                    channel_multiplier=-1)
                            ets.append((ki, et, lo))
                            if special:
                                want = min(n_av, 2 * (ki + 1))
                            else:
                                want = (n_av * (ki + 1)) // n_ki
                            while av_done < want:
                                emit_av(prev_sw, av_done)
                                av_done += 1
                            if special and av_done == n_av and ki >= 10:
                                if sp_state is None:
                                    sp_norm = dens(prev_sw)
                                    sp_state = {
                                        "qc": qc, "hp": hp, "n_ki": n_ki,
                                        "ets": ets,
                                        "pave": ps.tile([65, QT_BLK], f32,
                                                        name="pav_e",
                                                        tag="pave", bufs=1),
                                        "pavo": ps.tile([128, QT_BLK], f32,
                                                        name="pav_o",
                                                        tag="pavo", bufs=1),
                                    }
                                while sp_done < max(0, len(ets) - 2):
                                    emit_av(sp_state, sp_done)
                                    sp_done += 1
                            want_f = (len(fill) * (ki + 1)) // n_ki
                            while f_done < want_f:
                                fill[f_done]()
                                f_done += 1
                        while av_done < n_av:
                            emit_av(prev_sw, av_done)
                            av_done += 1
                        while f_done < len(fill):
                            fill[f_done]()
                            f_done += 1
                        if special:
                            new_norm = sp_norm
                        else:
                            new_norm = dens(prev_sw) if prev_sw else None
                        if pend_norm is not None:
                            bcast_norm(pend_norm)
                            if pend_norm[1] == 1:
                                wo_ready.append(pend_norm[0])
                        pend_norm = new_norm
                        if hp == 0:
                            if qc + 2 < N_QC:
                                dma_chunk(qc + 2)
                            if qc < 2:
                                proj(qc + 1)
                        if special:
                            sp_state["done"] = sp_done
                            prev_sw = sp_state
                        else:
                            prev_sw = {"qc": qc, "hp": hp, "n_ki": n_ki,
                                       "ets": ets}

                # flush: remaining AVs of the last sweep + trailing norms/Wo
                if "pave" not in prev_sw:
                    prev_sw["pave"] = ps.tile([65, QT_BLK], f32,
                                              name="pav_e", tag="pave",
                                              bufs=1)
                    prev_sw["pavo"] = ps.tile([128, QT_BLK], f32,
                                              name="pav_o", tag="pavo",
                                              bufs=1)
                for idx in range(prev_sw.get("done", 0),
                                 len(prev_sw["ets"])):
                    emit_av(prev_sw, idx)
                last_norm = dens(prev_sw)
                bcast_norm(pend_norm)  # (3, 0)
                # wo(2) split around the final normalize so its matmuls fill
                # the PE while the (3,1) reciprocal chain runs on DVE
                wo(2, jts=range(0, 4), dma_split=4)
                bcast_norm(last_norm)  # (3, 1)
                wo(2, jts=range(4, 8), bias_split=True, dma_split=4)
                wo(3, bias_split=True, dma_split=4)

    nc.compile()
    return nc


def kernel(query, key, value, mask, Wq, bq, Wk, bk, Wv, bv, Wo, bo):
    query = np.asarray(query, np.float32)
    key_ = np.asarray(key, np.float32)
    value = np.asarray(value, np.float32)
    Wq, Wk, Wv, Wo = (np.asarray(w, np.float32) for w in (Wq, Wk, Wv, Wo))
    bq, bk, bv, bo = (np.asarray(b_, np.float32) for b_ in (bq, bk, bv, bo))

    mask = np.asarray(mask)
    assert np.array_equal(mask != 0, np.tril(np.ones((S, S), bool))), \
        "kernel is specialized to the causal mask"
    if "nc" not in _CACHE:
        _CACHE["nc"] = _build()
    nc = _CACHE["nc"]

    def xt(x):  # [S, D] -> [N_QC, 128, KC, QT_BLK] bf16, partition-major
        a = x.T.reshape(KC, 128, S).transpose(1, 0, 2)  # [128, KC, S]
        a = a.reshape(128, KC, N_QC, QT_BLK).transpose(2, 0, 1, 3)
        return np.ascontiguousarray(a).astype(BF16)

    def wslice(W, c):  # [D, D] -> [128, KC, O] bf16 of W[o_slice].T
        hg = c % TP
        a = W[hg * O:(hg + 1) * O].T.reshape(KC, 128, O).transpose(1, 0, 2)
        return np.ascontiguousarray(a).astype(BF16)

    # bcast selector: rows 0-63 of pbc take 1/den_even (rdsb row 64),
    # rows 64-127 take 1/den_odd (rdsb row 0)
    bsel = np.zeros((65, 128), np.float32)
    bsel[64, 0:64] = 1.0
    bsel[0, 64:128] = 1.0

    in_maps = []
    for c in range(N_CORES):
        b_, hg = c // TP, c % TP
        osl = slice(hg * O, (hg + 1) * O)
        bo_part = bo if hg == 0 else np.zeros_like(bo)
        wot = Wo[:, osl].T.reshape(2, 128, D).transpose(1, 0, 2)
        in_maps.append({
            "xqt": xt(query[b_]),
            "xkt": xt(key_[b_]),
            "xvt": xt(value[b_]),
            "wqt": wslice(Wq, c),
            "wkt": wslice(Wk, c),
            "wvt": wslice(Wv, c),
            "wot": np.ascontiguousarray(wot).astype(BF16),
            "bqc": np.ascontiguousarray(bq[osl].reshape(2, 128).T),
            "bkc": np.ascontiguousarray(bk[osl].reshape(2, 128).T),
            "bvb": np.ascontiguousarray(np.broadcast_to(bv[osl], (128, O))),
            "boc": np.ascontiguousarray(bo_part.reshape(8, 128).T),
            "bsel": bsel,
        })

    res = run_bass_kernel_spmd(nc, in_maps, core_ids=list(range(N_CORES)))

    out = np.zeros((B, S, D), np.float32)
    for c in range(N_CORES):
        part = res.results[c]["out"].reshape(D, S)  # out^T [j, s]
        out[c // TP] += part.T.astype(np.float32)
    return out


# revision 63
# speedup vs baseline: 1.1664x; 1.1664x over previous
"""Trainium2 Bass kernel for nn_MultiHeadAttention_76295799046818.

MHA: B=2, S=2048, D=1024, H=16 heads (d_k=64), causal, fp32 reference.
Sharded over 8 NeuronCores: data-parallel over batch (2) x tensor-parallel
over heads (4 heads/core).  Wq/Wk/Wv column-parallel; Wo row-parallel with
the 4 partial outputs per batch summed on the host (cheaper than an
on-device all-reduce in this runtime).

Single fused pipeline per core (bf16 matmuls, fp32 PSUM):
  - Projections are chunked by 512 rows of S and interleaved with attention:
    chunk sc feeds attention q-chunk qc=sc, so QK/exp start ~10us in while
    later chunks still stream from HBM.
  - Per (head-pair hp, q-chunk qc) sweep: QK + exp per k-tile, with the
    sweep's AV matmuls DEFERRED one sweep and interleaved into the next
    sweep's k-tile loop (keeps the PE stream dense and gives the slow
    gpsimd semaphore hop a whole sweep of slack).  The pair's two heads
    sit at SBUF partitions 0-63/64-127 so their K=64 QK matmuls run on
    disjoint PE row groups concurrently.  exp is sliced to live columns;
    the causal staircase inside diagonal tiles is zeroed by gpsimd
    affine_select on the bf16 exp tile (no PE mask matmuls, no mask DMA).
  - AV accumulates [V|ones] / [ones|0|V] weights so softmax denominators
    land at PSUM rows 64 (even head) / 0 (odd head); C^T raw is copied out
    bf16 immediately (frees PSUM); the raw denominators are broadcast
    across partitions by a one-hot PE matmul, reciprocal'd with DVE
    reciprocal_approx_fast (only works on partition-base-0 SBUF APs), and
    a DVE multiply normalizes - trailing the sweep by one more slot.
  - Wo partials per q-chunk trail two slots behind, interleaved as PE
    filler units (with next-chunk projections) into the Scalar-paced
    hp=0 sweeps; output bias via Scalar ACT, bf16 DMA out.
"""

import numpy as np
import ml_dtypes

import concourse.bass as bass
import concourse.mybir as mybir
import concourse.tile as tile
from concourse import bacc
from concourse.bass_utils import run_bass_kernel_spmd

BF16 = ml_dtypes.bfloat16

B, S, D, H, DK = 2, 2048, 1024, 16, 64
N_CORES = 8
TP = 4  # head-parallel degree (per batch)
HPC = H // TP  # heads per core = 4
O = HPC * DK  # output channels per core = 256
QT_BLK = 512
N_QC = S // QT_BLK  # 4
KC = D // 128  # 8 contraction chunks for projections

_CACHE = {}


def _build():
    nc = bacc.Bacc("TRN2", target_bir_lowering=False, debug=False,
                   num_devices=N_CORES)
    dt = mybir.dt
    f32, bf16, f32r = dt.float32, dt.bfloat16, dt.float32r

    def din(name, shape, dtype=bf16):
        return nc.dram_tensor(name, shape, dtype, kind="ExternalInput").ap()

    xqt_d = din("xqt", [N_QC, 128, KC, QT_BLK])
    xkt_d = din("xkt", [N_QC, 128, KC, QT_BLK])
    xvt_d = din("xvt", [N_QC, 128, KC, QT_BLK])
    wqt_d = din("wqt", [128, KC, O])
    wkt_d = din("wkt", [128, KC, O])
    wvt_d = din("wvt", [128, KC, O])
    wot_d = din("wot", [128, 2, D])
    bq_d = din("bqc", [128, 2], f32)
    bk_d = din("bkc", [128, 2], f32)
    bvb_d = din("bvb", [128, O], f32)
    bo_d = din("boc", [128, 8], f32)
    bsel_d = din("bsel", [65, 128], f32r)
    out_d = nc.dram_tensor("out", [8, 128, S], bf16,
                           kind="ExternalOutput").ap()

    EXPF = mybir.ActivationFunctionType.Exp
    IDF = mybir.ActivationFunctionType.Identity

    with tile.TileContext(nc) as tc:
        with (
            tc.tile_pool(name="const", bufs=1) as cpool,
            tc.tile_pool(name="xin", bufs=2) as xpool,
            tc.tile_pool(name="expp", bufs=34) as epool,
            tc.tile_pool(name="crp", bufs=3) as crpool,
            tc.tile_pool(name="ctp", bufs=2) as ctpool,
            tc.tile_pool(name="outp", bufs=4) as opool,
        ):
            # hot-path weights + first x chunks, in priority order: the K
            # projection of chunk 0 gates everything, so its DMAs go first
            # smallest possible first slices so the K chain's kc=0 matmul
            # fires as early as the DMA pipe allows
            wk_sb = cpool.tile([128, KC, O], bf16, name="wk_sb")
            nc.sync.dma_start(wk_sb[:, 0:2, :], wkt_d[:, 0:2, :])
            nc.sync.dma_start(wk_sb[:, 2:8, :], wkt_d[:, 2:8, :])
            xk0 = xpool.tile([128, KC, QT_BLK], bf16, name="xk", tag="xk")
            nc.sync.dma_start(xk0[:, 0:1, :], xkt_d[0][:, 0:1, :])
            nc.sync.dma_start(xk0[:, 1:2, :], xkt_d[0][:, 1:2, :])
            nc.sync.dma_start(xk0[:, 2:4, :], xkt_d[0][:, 2:4, :])
            nc.sync.dma_start(xk0[:, 4:6, :], xkt_d[0][:, 4:6, :])
            nc.sync.dma_start(xk0[:, 6:8, :], xkt_d[0][:, 6:8, :])
            wq_sb = cpool.tile([128, KC, O], bf16, name="wq_sb")
            nc.sync.dma_start(wq_sb[:, 0:4, :], wqt_d[:, 0:4, :])
            nc.sync.dma_start(wq_sb[:, 4:8, :], wqt_d[:, 4:8, :])
            xq0 = xpool.tile([128, KC, QT_BLK], bf16, name="xq", tag="xq")
            nc.sync.dma_start(xq0[:, 0:4, :], xqt_d[0][:, 0:4, :])
            nc.sync.dma_start(xq0[:, 4:8, :], xqt_d[0][:, 4:8, :])
            wv_sb = cpool.tile([128, KC, O], bf16, name="wv_sb")
            nc.sync.dma_start(wv_sb[:, 0:4, :], wvt_d[:, 0:4, :])
            nc.sync.dma_start(wv_sb[:, 4:8, :], wvt_d[:, 4:8, :])
            xv0 = xpool.tile([128, KC, QT_BLK], bf16, name="xv", tag="xv")
            nc.sync.dma_start(xv0[:, 0:4, :], xvt_d[0][:, 0:4, :])
            nc.sync.dma_start(xv0[:, 4:8, :], xvt_d[0][:, 4:8, :])
            bq_sb = cpool.tile([128, 2], f32, name="bq_sb")
            nc.sync.dma_start(bq_sb[:], bq_d[:])
            bk_sb = cpool.tile([128, 2], f32, name="bk_sb")
            nc.sync.dma_start(bk_sb[:], bk_d[:])
            bvb_sb = cpool.tile([128, O], f32, name="bvb_sb")
            nc.sync.dma_start(bvb_sb[:], bvb_d[:])

            qt_sb = cpool.tile([128, 2, S], bf16, name="qt_sb")
            kt_sb = cpool.tile([128, 2, S], bf16, name="kt_sb")
            # AV weights: per k-tile/pair, even head [V|ones] (den @ row 64),
            # odd head [ones|0|V] (den @ row 0, C^T @ rows 64-127)
            vaug_e = cpool.tile([128, 16, 2, 66], bf16, name="vaug_e")
            nc.vector.memset(vaug_e[:], 1.0)
            vaug_o = cpool.tile([128, 16, 2, 128], bf16, name="vaug_o")
            nc.vector.memset(vaug_o[:], 0.0)
            nc.vector.memset(vaug_o[:, :, :, 0:1], 1.0)
            # raw-denominator staging rows 0 (odd head) / 64 (even head);
            # rows 1-63 stay 1.0 (multiplied by bsel zeros in the bcast).
            # memset can't emit f32r, so memset f32 and cast-copy once.
            onesf = cpool.tile([65, QT_BLK], f32, name="onesf")
            nc.vector.memset(onesf[:], 1.0)
            dsb = cpool.tile([65, 2, QT_BLK], f32r, name="dsb")
            nc.vector.tensor_copy(dsb[:, 0, :], onesf[:])
            nc.vector.tensor_copy(dsb[:, 1, :], onesf[:])

            xq_t, xk_t, xv_t = {}, {}, {}

            def dma_chunk(sc, eng=None):
                eng = eng or nc.sync
                xk = xpool.tile([128, KC, QT_BLK], bf16, name="xk", tag="xk")
                eng.dma_start(xk[:, 0:4, :], xkt_d[sc][:, 0:4, :])
                eng.dma_start(xk[:, 4:8, :], xkt_d[sc][:, 4:8, :])
                xq = xpool.tile([128, KC, QT_BLK], bf16, name="xq", tag="xq")
                eng.dma_start(xq[:, 0:4, :], xqt_d[sc][:, 0:4, :])
                eng.dma_start(xq[:, 4:8, :], xqt_d[sc][:, 4:8, :])
                xv = xpool.tile([128, KC, QT_BLK], bf16, name="xv", tag="xv")
                eng.dma_start(xv[:, 0:4, :], xvt_d[sc][:, 0:4, :])
                eng.dma_start(xv[:, 4:8, :], xvt_d[sc][:, 4:8, :])
                xk_t[sc], xq_t[sc], xv_t[sc] = xk, xq, xv

            with tc.tile_pool(name="ps", bufs=2, space="PSUM") as ps:

                def proj_units(sc):
                    """Projection for chunk sc as a list of small emission
                    units (a few matmuls each) so they can be interleaved
                    into attention sweeps as PE filler."""
                    ssl = bass.ds(sc * QT_BLK, QT_BLK)
                    xk, xq, xv = xk_t[sc], xq_t[sc], xv_t[sc]
                    units = []

                    def chain(w_sb, x, b_sb, dst_tile, dst_hp, ot):
                        box = {}

                        def u1():
                            box["p"] = ps.tile([128, QT_BLK], f32, name="pp",
                                               tag="big", bufs=2)
                            for kc in range(4):
                                nc.tensor.matmul(
                                    box["p"][:],
                                    w_sb[:, kc, bass.ds(ot * 128, 128)],
                                    x[:, kc, :], start=(kc == 0), stop=False,
                                    skip_group_check=True)

                        def u2():
                            for kc in range(4, KC):
                                nc.tensor.matmul(
                                    box["p"][:],
                                    w_sb[:, kc, bass.ds(ot * 128, 128)],
                                    x[:, kc, :], start=False,
                                    stop=(kc == KC - 1),
                                    skip_group_check=True)
                            nc.scalar.activation(dst_tile[:, dst_hp, ssl],
                                                 box["p"][:], IDF,
                                                 bias=b_sb[:, ot:ot + 1])
                        units.extend([u1, u2])

                    for ot in range(2):
                        chain(wk_sb, xk, bk_sb, kt_sb, ot, ot)
                    for ot in range(2):
                        chain(wq_sb, xq, bq_sb, qt_sb, ot, ot)

                    def vchain(mtp):
                        box = {}

                        def u1():
                            box["p"] = ps.tile([128, QT_BLK], f32, name="pp",
                                               tag="big", bufs=2)
                            for kc in range(KC):
                                nc.tensor.matmul(
                                    box["p"][:, 0:O],
                                    xv[:, kc, bass.ds(2 * mtp * 128, 128)],
                                    wv_sb[:, kc, :], start=(kc == 0),
                                    stop=(kc == KC - 1),
                                    skip_group_check=True)

                        def u2():
                            for kc in range(KC):
                                nc.tensor.matmul(
                                    box["p"][:, O:2 * O],
                                    xv[:, kc,
                                       bass.ds((2 * mtp + 1) * 128, 128)],
                                    wv_sb[:, kc, :], start=(kc == 0),
                                    stop=(kc == KC - 1),
                                    skip_group_check=True)
                            pv = box["p"]
                            pvr = pv[:].rearrange(
                                "p (mt hp two d) -> p mt hp two d",
                                mt=2, hp=2, two=2)
                            bvr = bvb_sb[:].rearrange(
                                "p (hp two d) -> p hp two d", hp=2, two=2)
                            for mt2 in range(2):
                                kt = sc * 4 + 2 * mtp + mt2
                                nc.vector.tensor_tensor(
                                    vaug_e[:, kt, :, 0:64],
                                    pvr[:, mt2, :, 0, :], bvr[:, :, 0, :],
                                    mybir.AluOpType.add)
                                nc.vector.tensor_tensor(
                                    vaug_o[:, kt, :, 64:128],
                                    pvr[:, mt2, :, 1, :], bvr[:, :, 1, :],
                                    mybir.AluOpType.add)
                        units.extend([u1, u2])

                    for mtp in range(2):
                        vchain(mtp)
                    return units

                def proj(sc, mid=None):
                    units = proj_units(sc)
                    for i, u in enumerate(units):
                        if i == 4 and mid is not None:
                            mid()
                        u()

                xk_t[0], xq_t[0], xv_t[0] = xk0, xq0, xv0
                # chunk-1 + tail constants dispatch from the Scalar DGE,
                # queued behind proj(0)'s K-bias ACTs, so the in-flight DMA
                # set stays small while xq0/xv0 stream (packets of all
                # in-flight DMAs share the queues round-robin)
                consts = {}

                def _lead_mid():
                    dma_chunk(1)
                    bsel_sb = cpool.tile([65, 128], f32r, name="bsel_sb")
                    nc.sync.dma_start(bsel_sb[:], bsel_d[:])
                    wo_sb = cpool.tile([128, 2, D], bf16, name="wo_sb")
                    nc.sync.dma_start(wo_sb[:, 0, :], wot_d[:, 0, :])
                    nc.sync.dma_start(wo_sb[:, 1, :], wot_d[:, 1, :])
                    bo_sb = cpool.tile([128, 8], f32, name="bo_sb")
                    nc.sync.dma_start(bo_sb[:], bo_d[:])
                    consts.update(bsel_sb=bsel_sb, wo_sb=wo_sb, bo_sb=bo_sb)

                proj(0, mid=_lead_mid)
                bsel_sb = consts["bsel_sb"]
                wo_sb = consts["wo_sb"]
                bo_sb = consts["bo_sb"]

                ct_t = {}
                prev = None

                def bcast_norm(state):
                    # broadcast raw dens across partitions (PE), reciprocal
                    # in SBUF at partition base 0 (recip_approx_fast only
                    # works there), then normalize ctraw -> ct
                    qc, hp, ctraw, pp = state
                    pdup = ps.tile([128, QT_BLK], f32, name="pdup",
                                   tag="big", bufs=2)
                    nc.tensor.matmul(pdup[:], bsel_sb[:], dsb[0:65, pp, :],
                                     start=True, stop=True)
                    pbcs = crpool.tile([128, QT_BLK], f32, name="pbcs",
                                       tag="pbcs")
                    nc.vector.tensor_copy(pbcs[:], pdup[:])
                    pbcr = crpool.tile([128, QT_BLK], f32, name="pbcr",
                                       tag="pbcr")
                    nc.vector.reciprocal_approx_fast(pbcr[:], pbcs[:])
                    if hp == 0:
                        ct = ctpool.tile([128, 2, QT_BLK], bf16, name="ct",
                                         tag="ct")
                        ct_t[qc] = ct
                    ct = ct_t[qc]
                    nc.vector.tensor_tensor(ct[:, hp, :], ctraw[:, :],
                                            pbcr[:, :], mybir.AluOpType.mult)

                def wo_units(qc, jts=range(8), bias_split=False,
                             dma_split=1):
                    ct = ct_t[qc]
                    units = []

                    def mk(jt):
                        def u():
                            pwo = ps.tile([128, QT_BLK], f32, name="pwo",
                                          tag="big", bufs=2)
                            for kc in range(2):
                                nc.tensor.matmul(
                                    pwo[:],
                                    wo_sb[:, kc, bass.ds(jt * 128, 128)],
                                    ct[:, kc, :], start=(kc == 0),
                                    stop=(kc == 1), skip_group_check=True)
                            if bias_split and jt % 2:
                                # DVE path uses its own pool tag: one writer
                                # engine per tag (mixed writers on one tag
                                # proved race-prone on this runtime)
                                osb = opool.tile([128, QT_BLK], bf16,
                                                 name="osbv", tag="osbv",
                                                 bufs=2)
                                nc.vector.tensor_scalar(
                                    osb[:], pwo[:], bo_sb[:, jt:jt + 1],
                                    None, mybir.AluOpType.add)
                            else:
                                osb = opool.tile([128, QT_BLK], bf16,
                                                 name="osb", tag="osb")
                                nc.scalar.activation(osb[:], pwo[:], IDF,
                                                     bias=bo_sb[:, jt:jt + 1])
                            # a single 128KB dma rides ~one queue (~27GB/s);
                            # split the drain-phase stores across queues so
                            # the end-of-kernel barrier isn't stuck on them
                            w = QT_BLK // dma_split
                            for sp in range(dma_split):
                                qs = bass.ds(qc * QT_BLK + sp * w, w)
                                nc.sync.dma_start(out_d[jt][:, qs],
                                                  osb[:, bass.ds(sp * w, w)])
                        return u

                    for jt in jts:
                        units.append(mk(jt))
                    return units

                def wo(qc, jts=range(8), bias_split=False, dma_split=1):
                    for u in wo_units(qc, jts, bias_split, dma_split):
                        u()

                # Sweep-level software pipeline: sweep N emits only QK+exp
                # (+affine); its AV matmuls are interleaved into sweep N+1's
                # k-tile loop.  This keeps the PE stream dense (long streaks
                # ramp the PE p-state) and gives the slow gpsimd affine hop a
                # whole sweep of slack before its AV consumer.
                def emit_av(s, idx):
                    ki, et, lo = s["ets"][idx]
                    alo = 0 if idx == 0 else lo
                    last = s["n_ki"] - 1
                    nc.tensor.matmul(
                        s["pave"][:, alo:QT_BLK],
                        vaug_e[:, ki, s["hp"], 0:65],
                        et[:, 0, alo:QT_BLK],
                        start=(idx == 0), stop=(idx == last),
                        skip_group_check=True)
                    nc.tensor.matmul(
                        s["pavo"][:, alo:QT_BLK],
                        vaug_o[:, ki, s["hp"], :],
                        et[:, 1, alo:QT_BLK],
                        start=(idx == 0), stop=(idx == last),
                        skip_group_check=True)

                def dens(s):
                    # raw denominators first (they gate the pdup broadcast),
                    # then raw C^T out of PSUM (frees pav)
                    pp = s["hp"]
                    nc.vector.tensor_copy(dsb[64:65, pp, :],
                                          s["pave"][64:65, :])
                    nc.vector.tensor_copy(dsb[0:1, pp, :],
                                          s["pavo"][0:1, :])
                    ctraw = crpool.tile([128, QT_BLK], bf16,
                                        name="ctraw", tag="ctraw")
                    nc.vector.tensor_copy(ctraw[0:64, :], s["pave"][0:64, :])
                    nc.vector.tensor_copy(ctraw[64:128, :],
                                          s["pavo"][64:128, :])
                    return (s["qc"], s["hp"], ctraw, pp)

                prev_sw = None   # sweep whose AVs run during current sweep
                pend_norm = None  # dens() output awaiting bcast_norm
                wo_ready = []    # qc values whose ct is fully normalized

                for qc in range(N_QC):
                    for hp in range(2):
                        n_ki = 4 * qc + 4
                        if prev_sw is not None:
                            prev_sw["pave"] = ps.tile([65, QT_BLK], f32,
                                                      name="pav_e",
                                                      tag="pave", bufs=1)
                            prev_sw["pavo"] = ps.tile([128, QT_BLK], f32,
                                                      name="pav_o",
                                                      tag="pavo", bufs=1)
                        n_av = len(prev_sw["ets"]) if prev_sw else 0
                        av_done = 0
                        # PE filler for Scalar-paced hp=0 sweeps: the next
                        # chunk's projections and the pending Wo, emitted in
                        # small units between k-tiles (their input data
                        # landed sweeps ago)
                        fill = []
                        if hp == 0 and qc >= 2:
                            if qc < 3:
                                fill += proj_units(qc + 1)
                            if wo_ready:
                                # bias_split: half the bias ops go to DVE so
                                # they don't sit in the Scalar stream ahead
                                # of this sweep's exps (Scalar paces hp=0
                                # sweeps)
                                fill += wo_units(wo_ready.pop(0),
                                                 bias_split=True)
                        f_done = 0
                        # last sweep: front-load the previous sweep's AVs,
                        # free its pav mid-sweep, then run our own AVs
                        # inline so the flush only drains a couple of tiles
                        # (tried: inline last-sweep AVs; regressed 27us -
                        # front-loaded AVs starve the st/exp pipeline)
                        special = False
                        sp_state = None
                        sp_norm = None
                        sp_done = 0
                        ets = []
                        for ki in range(n_ki):
                            lo = max(0, 128 * ki - QT_BLK * qc)
                            st = ps.tile([128, 2, QT_BLK], f32, name="st",
                                         tag="st", bufs=2)
                            for side in range(2):
                                po = bass.ds(side * 64, 64)
                                nc.tensor.matmul(
                                    st[:, side, lo:QT_BLK],
                                    kt_sb[po, hp, bass.ds(ki * 128, 128)],
                                    qt_sb[po, hp,
                                          bass.ds(qc * QT_BLK + lo,
                                                  QT_BLK - lo)],
                                    start=True, stop=True)
                            et = epool.tile([128, 2, QT_BLK], bf16,
                                            name="et", tag="et")
                            nc.scalar.activation(et[:, :, lo:QT_BLK],
                                                 st[:, :, lo:QT_BLK], EXPF,
                                                 scale=0.125)
                            if ki >= 4 * qc:
                                # zero the causal staircase (cols lo..lo+127)
                                nc.gpsimd.affine_select(
                                    out=et[:, :, bass.ds(lo, 128)],
                                    in_=et[:, :, bass.ds(lo, 128)],
                                    compare_op=mybir.AluOpType.is_ge,
                                    fill=0.0, base=0,
                                    pattern=[[0, 2], [1, 128]],
                                    channel_multiplier=-1)
                            ets.append((ki, et, lo))
                            if special:
                                want = min(n_av, 2 * (ki + 1))
                            else:
                                want = (n_av * (ki + 1)) // n_ki
                            while av_done < want:
                                emit_av(prev_sw, av_done)
                                av_done += 1
                            if special and av_done == n_av and ki >= 10:
                                if sp_state is None:
                                    sp_norm = dens(prev_sw)
                                    sp_state = {
                                        "qc": qc, "hp": hp, "n_ki": n_ki,
                                        "ets": ets,
                                        "pave": ps.tile([65, QT_BLK], f32,
                                                        name="pav_e",
                                                        tag="pave", bufs=1),
                                        "pavo": ps.tile([128, QT_BLK], f32,
                                                        name="pav_o",
                                                        tag="pavo", bufs=1),
                                    }
                                while sp_done < max(0, len(ets) - 2):
                                    emit_av(sp_state, sp_done)
                                    sp_done += 1
                            want_f = (len(fill) * (ki + 1)) // n_ki
                            while f_done < want_f:
                                fill[f_done]()
                                f_done += 1
                        while av_done < n_av:
                            emit_av(prev_sw, av_done)
                            av_done += 1
                        while f_done < len(fill):
                            fill[f_done]()
                            f_done += 1
                        if special:
                            new_norm = sp_norm
                        else:
                            new_norm = dens(prev_sw) if prev_sw else None
                        if pend_norm is not None:
                            bcast_norm(pend_norm)
                            if pend_norm[1] == 1:
                                wo_ready.append(pend_norm[0])
                        pend_norm = new_norm
                        if hp == 0:
                            if qc + 2 < N_QC:
                                dma_chunk(qc + 2)
                            if qc < 2:
                                proj(qc + 1)
                        if special:
                            sp_state["done"] = sp_done
                            prev_sw = sp_state
                        else:
                            prev_sw = {"qc": qc, "hp": hp, "n_ki": n_ki,
                                       "ets": ets}

                # flush: remaining AVs of the last sweep + trailing norms/Wo
                if "pave" not in prev_sw:
                    prev_sw["pave"] = ps.tile([65, QT_BLK], f32,
                                              name="pav_e", tag="pave",
                                              bufs=1)
                    prev_sw["pavo"] = ps.tile([128, QT_BLK], f32,
                                              name="pav_o", tag="pavo",
                                              bufs=1)
                for idx in range(prev_sw.get("done", 0),
                                 len(prev_sw["ets"])):
                    emit_av(prev_sw, idx)
                last_norm = dens(prev_sw)
                bcast_norm(pend_norm)  # (3, 0)
                # wo(2) split around the final normalize so its matmuls fill
                # the PE while the (3,1) reciprocal chain runs on DVE
                # (dma_split > 1 regressed: column-sliced stores drop the
                # per-partition contiguous run to 256B, halving DMA
                # efficiency - keep whole-tile stores)
                wo(2, jts=range(0, 4))
                bcast_norm(last_norm)  # (3, 1)
                wo(2, jts=range(4, 8), bias_split=True)
                wo(3, bias_split=True)

    nc.compile()
    return nc


def kernel(query, key, value, mask, Wq, bq, Wk, bk, Wv, bv, Wo, bo):
    query = np.asarray(query, np.float32)
    key_ = np.asarray(key, np.float32)
    value = np.asarray(value, np.float32)
    Wq, Wk, Wv, Wo = (np.asarray(w, np.float32) for w in (Wq, Wk, Wv, Wo))
    bq, bk, bv, bo = (np.asarray(b_, np.float32) for b_ in (bq, bk, bv, bo))

    mask = np.asarray(mask)
    assert np.array_equal(mask != 0, np.tril(np.ones((S, S), bool))), \
        "kernel is specialized to the causal mask"
    if "nc" not in _CACHE:
        _CACHE["nc"] = _build()
    nc = _CACHE["nc"]

    def xt(x):  # [S, D] -> [N_QC, 128, KC, QT_BLK] bf16, partition-major
        a = x.T.reshape(KC, 128, S).transpose(1, 0, 2)  # [128, KC, S]
        a = a.reshape(128, KC, N_QC, QT_BLK).transpose(2, 0, 1, 3)
        return np.ascontiguousarray(a).astype(BF16)

    def wslice(W, c):  # [D, D] -> [128, KC, O] bf16 of W[o_slice].T
        hg = c % TP
        a = W[hg * O:(hg + 1) * O].T.reshape(KC, 128, O).transpose(1, 0, 2)
        return np.ascontiguousarray(a).astype(BF16)

    # bcast selector: rows 0-63 of pbc take 1/den_even (rdsb row 64),
    # rows 64-127 take 1/den_odd (rdsb row 0)
    bsel = np.zeros((65, 128), np.float32)
    bsel[64, 0:64] = 1.0
    bsel[0, 64:128] = 1.0

    in_maps = []
    for c in range(N_CORES):
        b_, hg = c // TP, c % TP
        osl = slice(hg * O, (hg + 1) * O)
        bo_part = bo if hg == 0 else np.zeros_like(bo)
        wot = Wo[:, osl].T.reshape(2, 128, D).transpose(1, 0, 2)
        in_maps.append({
            "xqt": xt(query[b_]),
            "xkt": xt(key_[b_]),
            "xvt": xt(value[b_]),
            "wqt": wslice(Wq, c),
            "wkt": wslice(Wk, c),
            "wvt": wslice(Wv, c),
            "wot": np.ascontiguousarray(wot).astype(BF16),
            "bqc": np.ascontiguousarray(bq[osl].reshape(2, 128).T),
            "bkc": np.ascontiguousarray(bk[osl].reshape(2, 128).T),
            "bvb": np.ascontiguousarray(np.broadcast_to(bv[osl], (128, O))),
            "boc": np.ascontiguousarray(bo_part.reshape(8, 128).T),
            "bsel": bsel,
        })

    res = run_bass_kernel_spmd(nc, in_maps, core_ids=list(range(N_CORES)))

    out = np.zeros((B, S, D), np.float32)
    for c in range(N_CORES):
        part = res.results[c]["out"].reshape(D, S)  # out^T [j, s]
        out[c // TP] += part.T.astype(np.float32)
    return out


# revision 64
# speedup vs baseline: 1.1714x; 1.0042x over previous
"""Trainium2 Bass kernel for nn_MultiHeadAttention_76295799046818.

MHA: B=2, S=2048, D=1024, H=16 heads (d_k=64), causal, fp32 reference.
Sharded over 8 NeuronCores: data-parallel over batch (2) x tensor-parallel
over heads (4 heads/core).  Wq/Wk/Wv column-parallel; Wo row-parallel with
the 4 partial outputs per batch summed on the host (cheaper than an
on-device all-reduce in this runtime).

Single fused pipeline per core (bf16 matmuls, fp32 PSUM):
  - Projections are chunked by 512 rows of S and interleaved with attention:
    chunk sc feeds attention q-chunk qc=sc, so QK/exp start ~10us in while
    later chunks still stream from HBM.
  - Per (head-pair hp, q-chunk qc) sweep: QK + exp per k-tile, with the
    sweep's AV matmuls DEFERRED one sweep and interleaved into the next
    sweep's k-tile loop (keeps the PE stream dense and gives the slow
    gpsimd semaphore hop a whole sweep of slack).  The pair's two heads
    sit at SBUF partitions 0-63/64-127 so their K=64 QK matmuls run on
    disjoint PE row groups concurrently.  exp is sliced to live columns;
    the causal staircase inside diagonal tiles is zeroed by gpsimd
    affine_select on the bf16 exp tile (no PE mask matmuls, no mask DMA).
  - AV accumulates [V|ones] / [ones|0|V] weights so softmax denominators
    land at PSUM rows 64 (even head) / 0 (odd head); C^T raw is copied out
    bf16 immediately (frees PSUM); the raw denominators are broadcast
    across partitions by a one-hot PE matmul, reciprocal'd with DVE
    reciprocal_approx_fast (only works on partition-base-0 SBUF APs), and
    a DVE multiply normalizes - trailing the sweep by one more slot.
  - Wo partials per q-chunk trail two slots behind, interleaved as PE
    filler units (with next-chunk projections) into the Scalar-paced
    hp=0 sweeps; output bias via Scalar ACT, bf16 DMA out.
"""

import numpy as np
import ml_dtypes

import concourse.bass as bass
import concourse.mybir as mybir
import concourse.tile as tile
from concourse import bacc
from concourse.bass_utils import run_bass_kernel_spmd

BF16 = ml_dtypes.bfloat16

B, S, D, H, DK = 2, 2048, 1024, 16, 64
N_CORES = 8
TP = 4  # head-parallel degree (per batch)
HPC = H // TP  # heads per core = 4
O = HPC * DK  # output channels per core = 256
QT_BLK = 512
N_QC = S // QT_BLK  # 4
KC = D // 128  # 8 contraction chunks for projections

_CACHE = {}


def _build():
    nc = bacc.Bacc("TRN2", target_bir_lowering=False, debug=False,
                   num_devices=N_CORES)
    dt = mybir.dt
    f32, bf16, f32r = dt.float32, dt.bfloat16, dt.float32r

    def din(name, shape, dtype=bf16):
        return nc.dram_tensor(name, shape, dtype, kind="ExternalInput").ap()

    xqt_d = din("xqt", [N_QC, 128, KC, QT_BLK])
    xkt_d = din("xkt", [N_QC, 128, KC, QT_BLK])
    xvt_d = din("xvt", [N_QC, 128, KC, QT_BLK])
    wqt_d = din("wqt", [128, KC, O])
    wkt_d = din("wkt", [128, KC, O])
    wvt_d = din("wvt", [128, KC, O])
    wot_d = din("wot", [128, 2, D])
    bq_d = din("bqc", [128, 2], f32)
    bk_d = din("bkc", [128, 2], f32)
    bvb_d = din("bvb", [128, O], f32)
    bo_d = din("boc", [128, 8], f32)
    bsel_d = din("bsel", [65, 128], f32r)
    out_d = nc.dram_tensor("out", [8, 128, S], bf16,
                           kind="ExternalOutput").ap()

    EXPF = mybir.ActivationFunctionType.Exp
    IDF = mybir.ActivationFunctionType.Identity

    with tile.TileContext(nc) as tc:
        with (
            tc.tile_pool(name="const", bufs=1) as cpool,
            tc.tile_pool(name="xin", bufs=2) as xpool,
            tc.tile_pool(name="expp", bufs=34) as epool,
            tc.tile_pool(name="crp", bufs=3) as crpool,
            tc.tile_pool(name="ctp", bufs=2) as ctpool,
            tc.tile_pool(name="outp", bufs=4) as opool,
        ):
            # hot-path weights + first x chunks, in priority order: the K
            # projection of chunk 0 gates everything, so its DMAs go first
            # smallest possible first slices so the K chain's kc=0 matmul
            # fires as early as the DMA pipe allows
            wk_sb = cpool.tile([128, KC, O], bf16, name="wk_sb")
            nc.sync.dma_start(wk_sb[:, 0:2, :], wkt_d[:, 0:2, :])
            nc.sync.dma_start(wk_sb[:, 2:8, :], wkt_d[:, 2:8, :])
            xk0 = xpool.tile([128, KC, QT_BLK], bf16, name="xk", tag="xk")
            nc.sync.dma_start(xk0[:, 0:1, :], xkt_d[0][:, 0:1, :])
            nc.sync.dma_start(xk0[:, 1:2, :], xkt_d[0][:, 1:2, :])
            nc.sync.dma_start(xk0[:, 2:4, :], xkt_d[0][:, 2:4, :])
            nc.sync.dma_start(xk0[:, 4:6, :], xkt_d[0][:, 4:6, :])
            nc.sync.dma_start(xk0[:, 6:8, :], xkt_d[0][:, 6:8, :])
            # fine slices: in-flight DMAs round-robin, so small slices land
            # early and the Q/V chains stream at kc granularity instead of
            # waiting for whole tensors
            wq_sb = cpool.tile([128, KC, O], bf16, name="wq_sb")
            nc.sync.dma_start(wq_sb[:, 0:2, :], wqt_d[:, 0:2, :])
            nc.sync.dma_start(wq_sb[:, 2:8, :], wqt_d[:, 2:8, :])
            xq0 = xpool.tile([128, KC, QT_BLK], bf16, name="xq", tag="xq")
            nc.sync.dma_start(xq0[:, 0:1, :], xqt_d[0][:, 0:1, :])
            nc.sync.dma_start(xq0[:, 1:2, :], xqt_d[0][:, 1:2, :])
            nc.sync.dma_start(xq0[:, 2:4, :], xqt_d[0][:, 2:4, :])
            nc.sync.dma_start(xq0[:, 4:8, :], xqt_d[0][:, 4:8, :])
            wv_sb = cpool.tile([128, KC, O], bf16, name="wv_sb")
            nc.sync.dma_start(wv_sb[:, 0:2, :], wvt_d[:, 0:2, :])
            nc.sync.dma_start(wv_sb[:, 2:8, :], wvt_d[:, 2:8, :])
            xv0 = xpool.tile([128, KC, QT_BLK], bf16, name="xv", tag="xv")
            nc.sync.dma_start(xv0[:, 0:1, :], xvt_d[0][:, 0:1, :])
            nc.sync.dma_start(xv0[:, 1:2, :], xvt_d[0][:, 1:2, :])
            nc.sync.dma_start(xv0[:, 2:4, :], xvt_d[0][:, 2:4, :])
            nc.sync.dma_start(xv0[:, 4:8, :], xvt_d[0][:, 4:8, :])
            bq_sb = cpool.tile([128, 2], f32, name="bq_sb")
            nc.sync.dma_start(bq_sb[:], bq_d[:])
            bk_sb = cpool.tile([128, 2], f32, name="bk_sb")
            nc.sync.dma_start(bk_sb[:], bk_d[:])
            bvb_sb = cpool.tile([128, O], f32, name="bvb_sb")
            nc.sync.dma_start(bvb_sb[:], bvb_d[:])

            qt_sb = cpool.tile([128, 2, S], bf16, name="qt_sb")
            kt_sb = cpool.tile([128, 2, S], bf16, name="kt_sb")
            # AV weights: per k-tile/pair, even head [V|ones] (den @ row 64),
            # odd head [ones|0|V] (den @ row 0, C^T @ rows 64-127)
            vaug_e = cpool.tile([128, 16, 2, 66], bf16, name="vaug_e")
            nc.vector.memset(vaug_e[:], 1.0)
            vaug_o = cpool.tile([128, 16, 2, 128], bf16, name="vaug_o")
            nc.vector.memset(vaug_o[:], 0.0)
            nc.vector.memset(vaug_o[:, :, :, 0:1], 1.0)
            # raw-denominator staging rows 0 (odd head) / 64 (even head);
            # rows 1-63 stay 1.0 (multiplied by bsel zeros in the bcast).
            # memset can't emit f32r, so memset f32 and cast-copy once.
            onesf = cpool.tile([65, QT_BLK], f32, name="onesf")
            nc.vector.memset(onesf[:], 1.0)
            dsb = cpool.tile([65, 2, QT_BLK], f32r, name="dsb")
            nc.vector.tensor_copy(dsb[:, 0, :], onesf[:])
            nc.vector.tensor_copy(dsb[:, 1, :], onesf[:])

            xq_t, xk_t, xv_t = {}, {}, {}

            def dma_chunk(sc, eng=None):
                eng = eng or nc.sync
                xk = xpool.tile([128, KC, QT_BLK], bf16, name="xk", tag="xk")
                eng.dma_start(xk[:, 0:4, :], xkt_d[sc][:, 0:4, :])
                eng.dma_start(xk[:, 4:8, :], xkt_d[sc][:, 4:8, :])
                xq = xpool.tile([128, KC, QT_BLK], bf16, name="xq", tag="xq")
                eng.dma_start(xq[:, 0:4, :], xqt_d[sc][:, 0:4, :])
                eng.dma_start(xq[:, 4:8, :], xqt_d[sc][:, 4:8, :])
                xv = xpool.tile([128, KC, QT_BLK], bf16, name="xv", tag="xv")
                eng.dma_start(xv[:, 0:4, :], xvt_d[sc][:, 0:4, :])
                eng.dma_start(xv[:, 4:8, :], xvt_d[sc][:, 4:8, :])
                xk_t[sc], xq_t[sc], xv_t[sc] = xk, xq, xv

            with tc.tile_pool(name="ps", bufs=2, space="PSUM") as ps:

                def proj_units(sc):
                    """Projection for chunk sc as a list of small emission
                    units (a few matmuls each) so they can be interleaved
                    into attention sweeps as PE filler."""
                    ssl = bass.ds(sc * QT_BLK, QT_BLK)
                    xk, xq, xv = xk_t[sc], xq_t[sc], xv_t[sc]
                    units = []

                    def chain(w_sb, x, b_sb, dst_tile, dst_hp, ot):
                        box = {}

                        def u1():
                            box["p"] = ps.tile([128, QT_BLK], f32, name="pp",
                                               tag="big", bufs=2)
                            for kc in range(4):
                                nc.tensor.matmul(
                                    box["p"][:],
                                    w_sb[:, kc, bass.ds(ot * 128, 128)],
                                    x[:, kc, :], start=(kc == 0), stop=False,
                                    skip_group_check=True)

                        def u2():
                            for kc in range(4, KC):
                                nc.tensor.matmul(
                                    box["p"][:],
                                    w_sb[:, kc, bass.ds(ot * 128, 128)],
                                    x[:, kc, :], start=False,
                                    stop=(kc == KC - 1),
                                    skip_group_check=True)
                            nc.scalar.activation(dst_tile[:, dst_hp, ssl],
                                                 box["p"][:], IDF,
                                                 bias=b_sb[:, ot:ot + 1])
                        units.extend([u1, u2])

                    for ot in range(2):
                        chain(wk_sb, xk, bk_sb, kt_sb, ot, ot)
                    for ot in range(2):
                        chain(wq_sb, xq, bq_sb, qt_sb, ot, ot)

                    def vchain(mtp):
                        box = {}

                        def u1():
                            box["p"] = ps.tile([128, QT_BLK], f32, name="pp",
                                               tag="big", bufs=2)
                            for kc in range(KC):
                                nc.tensor.matmul(
                                    box["p"][:, 0:O],
                                    xv[:, kc, bass.ds(2 * mtp * 128, 128)],
                                    wv_sb[:, kc, :], start=(kc == 0),
                                    stop=(kc == KC - 1),
                                    skip_group_check=True)

                        def u2():
                            for kc in range(KC):
                                nc.tensor.matmul(
                                    box["p"][:, O:2 * O],
                                    xv[:, kc,
                                       bass.ds((2 * mtp + 1) * 128, 128)],
                                    wv_sb[:, kc, :], start=(kc == 0),
                                    stop=(kc == KC - 1),
                                    skip_group_check=True)
                            pv = box["p"]
                            pvr = pv[:].rearrange(
                                "p (mt hp two d) -> p mt hp two d",
                                mt=2, hp=2, two=2)
                            bvr = bvb_sb[:].rearrange(
                                "p (hp two d) -> p hp two d", hp=2, two=2)
                            for mt2 in range(2):
                                kt = sc * 4 + 2 * mtp + mt2
                                nc.vector.tensor_tensor(
                                    vaug_e[:, kt, :, 0:64],
                                    pvr[:, mt2, :, 0, :], bvr[:, :, 0, :],
                                    mybir.AluOpType.add)
                                nc.vector.tensor_tensor(
                                    vaug_o[:, kt, :, 64:128],
                                    pvr[:, mt2, :, 1, :], bvr[:, :, 1, :],
                                    mybir.AluOpType.add)
                        units.extend([u1, u2])

                    for mtp in range(2):
                        vchain(mtp)
                    return units

                def proj(sc, mid=None):
                    units = proj_units(sc)
                    for i, u in enumerate(units):
                        if i == 4 and mid is not None:
                            mid()
                        u()

                xk_t[0], xq_t[0], xv_t[0] = xk0, xq0, xv0
                # chunk-1 + tail constants dispatch from the Scalar DGE,
                # queued behind proj(0)'s K-bias ACTs, so the in-flight DMA
                # set stays small while xq0/xv0 stream (packets of all
                # in-flight DMAs share the queues round-robin)
                consts = {}

                def _lead_mid():
                    dma_chunk(1)
                    bsel_sb = cpool.tile([65, 128], f32r, name="bsel_sb")
                    nc.sync.dma_start(bsel_sb[:], bsel_d[:])
                    wo_sb = cpool.tile([128, 2, D], bf16, name="wo_sb")
                    nc.sync.dma_start(wo_sb[:, 0, :], wot_d[:, 0, :])
                    nc.sync.dma_start(wo_sb[:, 1, :], wot_d[:, 1, :])
                    bo_sb = cpool.tile([128, 8], f32, name="bo_sb")
                    nc.sync.dma_start(bo_sb[:], bo_d[:])
                    consts.update(bsel_sb=bsel_sb, wo_sb=wo_sb, bo_sb=bo_sb)

                proj(0, mid=_lead_mid)
                bsel_sb = consts["bsel_sb"]
                wo_sb = consts["wo_sb"]
                bo_sb = consts["bo_sb"]

                ct_t = {}
                prev = None

                def bcast_norm(state):
                    # broadcast raw dens across partitions (PE), reciprocal
                    # in SBUF at partition base 0 (recip_approx_fast only
                    # works there), then normalize ctraw -> ct
                    qc, hp, ctraw, pp = state
                    pdup = ps.tile([128, QT_BLK], f32, name="pdup",
                                   tag="big", bufs=2)
                    nc.tensor.matmul(pdup[:], bsel_sb[:], dsb[0:65, pp, :],
                                     start=True, stop=True)
                    pbcs = crpool.tile([128, QT_BLK], f32, name="pbcs",
                                       tag="pbcs")
                    nc.vector.tensor_copy(pbcs[:], pdup[:])
                    pbcr = crpool.tile([128, QT_BLK], f32, name="pbcr",
                                       tag="pbcr")
                    nc.vector.reciprocal_approx_fast(pbcr[:], pbcs[:])
                    if hp == 0:
                        ct = ctpool.tile([128, 2, QT_BLK], bf16, name="ct",
                                         tag="ct")
                        ct_t[qc] = ct
                    ct = ct_t[qc]
                    nc.vector.tensor_tensor(ct[:, hp, :], ctraw[:, :],
                                            pbcr[:, :], mybir.AluOpType.mult)

                def wo_units(qc, jts=range(8), bias_split=False,
                             dma_split=1):
                    ct = ct_t[qc]
                    units = []

                    def mk(jt):
                        def u():
                            pwo = ps.tile([128, QT_BLK], f32, name="pwo",
                                          tag="big", bufs=2)
                            for kc in range(2):
                                nc.tensor.matmul(
                                    pwo[:],
                                    wo_sb[:, kc, bass.ds(jt * 128, 128)],
                                    ct[:, kc, :], start=(kc == 0),
                                    stop=(kc == 1), skip_group_check=True)
                            if bias_split and jt % 2:
                                # DVE path uses its own pool tag: one writer
                                # engine per tag (mixed writers on one tag
                                # proved race-prone on this runtime)
                                osb = opool.tile([128, QT_BLK], bf16,
                                                 name="osbv", tag="osbv",
                                                 bufs=2)
                                nc.vector.tensor_scalar(
                                    osb[:], pwo[:], bo_sb[:, jt:jt + 1],
                                    None, mybir.AluOpType.add)
                            else:
                                osb = opool.tile([128, QT_BLK], bf16,
                                                 name="osb", tag="osb")
                                nc.scalar.activation(osb[:], pwo[:], IDF,
                                                     bias=bo_sb[:, jt:jt + 1])
                            # a single 128KB dma rides ~one queue (~27GB/s);
                            # split the drain-phase stores across queues so
                            # the end-of-kernel barrier isn't stuck on them
                            w = QT_BLK // dma_split
                            for sp in range(dma_split):
                                qs = bass.ds(qc * QT_BLK + sp * w, w)
                                nc.sync.dma_start(out_d[jt][:, qs],
                                                  osb[:, bass.ds(sp * w, w)])
                        return u

                    for jt in jts:
                        units.append(mk(jt))
                    return units

                def wo(qc, jts=range(8), bias_split=False, dma_split=1):
                    for u in wo_units(qc, jts, bias_split, dma_split):
                        u()

                # Sweep-level software pipeline: sweep N emits only QK+exp
                # (+affine); its AV matmuls are interleaved into sweep N+1's
                # k-tile loop.  This keeps the PE stream dense (long streaks
                # ramp the PE p-state) and gives the slow gpsimd affine hop a
                # whole sweep of slack before its AV consumer.
                def emit_av(s, idx):
                    ki, et, lo = s["ets"][idx]
                    alo = 0 if idx == 0 else lo
                    last = s["n_ki"] - 1
                    nc.tensor.matmul(
                        s["pave"][:, alo:QT_BLK],
                        vaug_e[:, ki, s["hp"], 0:65],
                        et[:, 0, alo:QT_BLK],
                        start=(idx == 0), stop=(idx == last),
                        skip_group_check=True)
                    nc.tensor.matmul(
                        s["pavo"][:, alo:QT_BLK],
                        vaug_o[:, ki, s["hp"], :],
                        et[:, 1, alo:QT_BLK],
                        start=(idx == 0), stop=(idx == last),
                        skip_group_check=True)

                def dens(s):
                    # raw denominators first (they gate the pdup broadcast),
                    # then raw C^T out of PSUM (frees pav)
                    pp = s["hp"]
                    nc.vector.tensor_copy(dsb[64:65, pp, :],
                                          s["pave"][64:65, :])
                    nc.vector.tensor_copy(dsb[0:1, pp, :],
                                          s["pavo"][0:1, :])
                    ctraw = crpool.tile([128, QT_BLK], bf16,
                                        name="ctraw", tag="ctraw")
                    nc.vector.tensor_copy(ctraw[0:64, :], s["pave"][0:64, :])
                    nc.vector.tensor_copy(ctraw[64:128, :],
                                          s["pavo"][64:128, :])
                    return (s["qc"], s["hp"], ctraw, pp)

                prev_sw = None   # sweep whose AVs run during current sweep
                pend_norm = None  # dens() output awaiting bcast_norm
                wo_ready = []    # qc values whose ct is fully normalized

                for qc in range(N_QC):
                    for hp in range(2):
                        n_ki = 4 * qc + 4
                        if prev_sw is not None:
                            prev_sw["pave"] = ps.tile([65, QT_BLK], f32,
                                                      name="pav_e",
                                                      tag="pave", bufs=1)
                            prev_sw["pavo"] = ps.tile([128, QT_BLK], f32,
                                                      name="pav_o",
                                                      tag="pavo", bufs=1)
                        n_av = len(prev_sw["ets"]) if prev_sw else 0
                        av_done = 0
                        # PE filler for Scalar-paced hp=0 sweeps: the next
                        # chunk's projections and the pending Wo, emitted in
                        # small units between k-tiles (their input data
                        # landed sweeps ago)
                        fill = []
                        if hp == 0 and qc >= 2:
                            if qc < 3:
                                fill += proj_units(qc + 1)
                            if wo_ready:
                                # bias_split: half the bias ops go to DVE so
                                # they don't sit in the Scalar stream ahead
                                # of this sweep's exps (Scalar paces hp=0
                                # sweeps)
                                fill += wo_units(wo_ready.pop(0),
                                                 bias_split=True)
                        f_done = 0
                        # last sweep: front-load the previous sweep's AVs,
                        # free its pav mid-sweep, then run our own AVs
                        # inline so the flush only drains a couple of tiles
                        # (tried: inline last-sweep AVs; regressed 27us -
                        # front-loaded AVs starve the st/exp pipeline)
                        special = False
                        sp_state = None
                        sp_norm = None
                        sp_done = 0
                        ets = []
                        for ki in range(n_ki):
                            lo = max(0, 128 * ki - QT_BLK * qc)
                            st = ps.tile([128, 2, QT_BLK], f32, name="st",
                                         tag="st", bufs=2)
                            for side in range(2):
                                po = bass.ds(side * 64, 64)
                                nc.tensor.matmul(
                                    st[:, side, lo:QT_BLK],
                                    kt_sb[po, hp, bass.ds(ki * 128, 128)],
                                    qt_sb[po, hp,
                                          bass.ds(qc * QT_BLK + lo,
                                                  QT_BLK - lo)],
                                    start=True, stop=True)
                            et = epool.tile([128, 2, QT_BLK], bf16,
                                            name="et", tag="et")
                            nc.scalar.activation(et[:, :, lo:QT_BLK],
                                                 st[:, :, lo:QT_BLK], EXPF,
                                                 scale=0.125)
                            if ki >= 4 * qc:
                                # zero the causal staircase (cols lo..lo+127)
                                nc.gpsimd.affine_select(
                                    out=et[:, :, bass.ds(lo, 128)],
                                    in_=et[:, :, bass.ds(lo, 128)],
                                    compare_op=mybir.AluOpType.is_ge,
                                    fill=0.0, base=0,
                                    pattern=[[0, 2], [1, 128]],
                                    channel_multiplier=-1)
                            ets.append((ki, et, lo))
                            if special:
                                want = min(n_av, 2 * (ki + 1))
                            else:
                                want = (n_av * (ki + 1)) // n_ki
                            while av_done < want:
                                emit_av(prev_sw, av_done)
                                av_done += 1
                            if special and av_done == n_av and ki >= 10:
                                if sp_state is None:
                                    sp_norm = dens(prev_sw)
                                    sp_state = {
                                        "qc": qc, "hp": hp, "n_ki": n_ki,
                                        "ets": ets,
                                        "pave": ps.tile([65, QT_BLK], f32,
                                                        name="pav_e",
                                                        tag="pave", bufs=1),
                                        "pavo": ps.tile([128, QT_BLK], f32,
                                                        name="pav_o",
                                                        tag="pavo", bufs=1),
                                    }
                                while sp_done < max(0, len(ets) - 2):
                                    emit_av(sp_state, sp_done)
                                    sp_done += 1
                            want_f = (len(fill) * (ki + 1)) // n_ki
                            while f_done < want_f:
                                fill[f_done]()
                                f_done += 1
                        while av_done < n_av:
                            emit_av(prev_sw, av_done)
                            av_done += 1
                        while f_done < len(fill):
                            fill[f_done]()
                            f_done += 1
                        if special:
                            new_norm = sp_norm
                        else:
                            new_norm = dens(prev_sw) if prev_sw else None
                        if pend_norm is not None:
                            bcast_norm(pend_norm)
                            if pend_norm[1] == 1:
                                wo_ready.append(pend_norm[0])
                        pend_norm = new_norm
                        if hp == 0:
                            if qc + 2 < N_QC:
                                dma_chunk(qc + 2)
                            if qc < 2:
                                proj(qc + 1)
                        if special:
                            sp_state["done"] = sp_done
                            prev_sw = sp_state
                        else:
                            prev_sw = {"qc": qc, "hp": hp, "n_ki": n_ki,
                                       "ets": ets}

                # flush: remaining AVs of the last sweep + trailing norms/Wo
                if "pave" not in prev_sw:
                    prev_sw["pave"] = ps.tile([65, QT_BLK], f32,
                                              name="pav_e", tag="pave",
                                              bufs=1)
                    prev_sw["pavo"] = ps.tile([128, QT_BLK], f32,
                                              name="pav_o", tag="pavo",
                                              bufs=1)
                for idx in range(prev_sw.get("done", 0),
                                 len(prev_sw["ets"])):
                    emit_av(prev_sw, idx)
                last_norm = dens(prev_sw)
                bcast_norm(pend_norm)  # (3, 0)
                # wo(2) split around the final normalize so its matmuls fill
                # the PE while the (3,1) reciprocal chain runs on DVE
                # (dma_split > 1 regressed: column-sliced stores drop the
                # per-partition contiguous run to 256B, halving DMA
                # efficiency - keep whole-tile stores)
                wo(2, jts=range(0, 4))
                bcast_norm(last_norm)  # (3, 1)
                wo(2, jts=range(4, 8), bias_split=True)
                wo(3, bias_split=True)

    nc.compile()
    return nc


def kernel(query, key, value, mask, Wq, bq, Wk, bk, Wv, bv, Wo, bo):
    query = np.asarray(query, np.float32)
    key_ = np.asarray(key, np.float32)
    value = np.asarray(value, np.float32)
    Wq, Wk, Wv, Wo = (np.asarray(w, np.float32) for w in (Wq, Wk, Wv, Wo))
    bq, bk, bv, bo = (np.asarray(b_, np.float32) for b_ in (bq, bk, bv, bo))

    mask = np.asarray(mask)
    assert np.array_equal(mask != 0, np.tril(np.ones((S, S), bool))), \
        "kernel is specialized to the causal mask"
    if "nc" not in _CACHE:
        _CACHE["nc"] = _build()
    nc = _CACHE["nc"]

    def xt(x):  # [S, D] -> [N_QC, 128, KC, QT_BLK] bf16, partition-major
        a = x.T.reshape(KC, 128, S).transpose(1, 0, 2)  # [128, KC, S]
        a = a.reshape(128, KC, N_QC, QT_BLK).transpose(2, 0, 1, 3)
        return np.ascontiguousarray(a).astype(BF16)

    def wslice(W, c):  # [D, D] -> [128, KC, O] bf16 of W[o_slice].T
        hg = c % TP
        a = W[hg * O:(hg + 1) * O].T.reshape(KC, 128, O).transpose(1, 0, 2)
        return np.ascontiguousarray(a).astype(BF16)

    # bcast selector: rows 0-63 of pbc take 1/den_even (rdsb row 64),
    # rows 64-127 take 1/den_odd (rdsb row 0)
    bsel = np.zeros((65, 128), np.float32)
    bsel[64, 0:64] = 1.0
    bsel[0, 64:128] = 1.0

    in_maps = []
    for c in range(N_CORES):
        b_, hg = c // TP, c % TP
        osl = slice(hg * O, (hg + 1) * O)
        bo_part = bo if hg == 0 else np.zeros_like(bo)
        wot = Wo[:, osl].T.reshape(2, 128, D).transpose(1, 0, 2)
        in_maps.append({
            "xqt": xt(query[b_]),
            "xkt": xt(key_[b_]),
            "xvt": xt(value[b_]),
            "wqt": wslice(Wq, c),
            "wkt": wslice(Wk, c),
            "wvt": wslice(Wv, c),
            "wot": np.ascontiguousarray(wot).astype(BF16),
            "bqc": np.ascontiguousarray(bq[osl].reshape(2, 128).T),
            "bkc": np.ascontiguousarray(bk[osl].reshape(2, 128).T),
            "bvb": np.ascontiguousarray(np.broadcast_to(bv[osl], (128, O))),
            "boc": np.ascontiguousarray(bo_part.reshape(8, 128).T),
            "bsel": bsel,
        })

    res = run_bass_kernel_spmd(nc, in_maps, core_ids=list(range(N_CORES)))

    out = np.zeros((B, S, D), np.float32)
    for c in range(N_CORES):
        part = res.results[c]["out"].reshape(D, S)  # out^T [j, s]
        out[c // TP] += part.T.astype(np.float32)
    return out


# revision 66
# speedup vs baseline: 1.2217x; 1.0430x over previous
"""Trainium2 Bass kernel for nn_MultiHeadAttention_76295799046818.

MHA: B=2, S=2048, D=1024, H=16 heads (d_k=64), causal, fp32 reference.
Sharded over 8 NeuronCores: data-parallel over batch (2) x tensor-parallel
over heads (4 heads/core).  Wq/Wk/Wv column-parallel; Wo row-parallel with
the 4 partial outputs per batch summed on the host (cheaper than an
on-device all-reduce in this runtime).

Single fused pipeline per core (bf16 matmuls, fp32 PSUM):
  - Projections are chunked by 512 rows of S and interleaved with attention:
    chunk sc feeds attention q-chunk qc=sc, so QK/exp start ~10us in while
    later chunks still stream from HBM.
  - Per (head-pair hp, q-chunk qc) sweep: QK + exp per k-tile, with the
    sweep's AV matmuls DEFERRED one sweep and interleaved into the next
    sweep's k-tile loop (keeps the PE stream dense and gives the slow
    gpsimd semaphore hop a whole sweep of slack).  The pair's two heads
    sit at SBUF partitions 0-63/64-127 so their K=64 QK matmuls run on
    disjoint PE row groups concurrently.  exp is sliced to live columns;
    the causal staircase inside diagonal tiles is zeroed by gpsimd
    affine_select on the bf16 exp tile (no PE mask matmuls, no mask DMA).
  - AV accumulates [V|ones] / [ones|0|V] weights so softmax denominators
    land at PSUM rows 64 (even head) / 0 (odd head); C^T raw is copied out
    bf16 immediately (frees PSUM); the raw denominators are broadcast
    across partitions by a one-hot PE matmul, reciprocal'd with DVE
    reciprocal_approx_fast (only works on partition-base-0 SBUF APs), and
    a DVE multiply normalizes - trailing the sweep by one more slot.
  - Wo partials per q-chunk trail two slots behind, interleaved as PE
    filler units (with next-chunk projections) into the Scalar-paced
    hp=0 sweeps; output bias via Scalar ACT, bf16 DMA out.
"""

import numpy as np
import ml_dtypes

import concourse.bass as bass
import concourse.mybir as mybir
import concourse.tile as tile
from concourse import bacc
from concourse.bass_utils import run_bass_kernel_spmd

BF16 = ml_dtypes.bfloat16

B, S, D, H, DK = 2, 2048, 1024, 16, 64
N_CORES = 8
TP = 4  # head-parallel degree (per batch)
HPC = H // TP  # heads per core = 4
O = HPC * DK  # output channels per core = 256
QT_BLK = 512
N_QC = S // QT_BLK  # 4
KC = D // 128  # 8 contraction chunks for projections

_CACHE = {}


def _build():
    nc = bacc.Bacc("TRN2", target_bir_lowering=False, debug=False,
                   num_devices=N_CORES)
    dt = mybir.dt
    f32, bf16, f32r = dt.float32, dt.bfloat16, dt.float32r

    def din(name, shape, dtype=bf16):
        return nc.dram_tensor(name, shape, dtype, kind="ExternalInput").ap()

    xqt_d = din("xqt", [N_QC, 128, KC, QT_BLK])
    xkt_d = din("xkt", [N_QC, 128, KC, QT_BLK])
    xvt_d = din("xvt", [N_QC, 128, KC, QT_BLK])
    wqt_d = din("wqt", [128, KC, O])
    wkt_d = din("wkt", [128, KC, O])
    wvt_d = din("wvt", [128, KC, O])
    wot_d = din("wot", [128, 2, D])
    bq_d = din("bqc", [128, 2], f32)
    bk_d = din("bkc", [128, 2], f32)
    bvb_d = din("bvb", [128, O], f32)
    bo_d = din("boc", [128, 8], f32)
    bsel_d = din("bsel", [65, 128], f32r)
    out_d = nc.dram_tensor("out", [8, 128, S], bf16,
                           kind="ExternalOutput").ap()

    EXPF = mybir.ActivationFunctionType.Exp
    IDF = mybir.ActivationFunctionType.Identity

    with tile.TileContext(nc) as tc:
        with (
            tc.tile_pool(name="const", bufs=1) as cpool,
            tc.tile_pool(name="xin", bufs=2) as xpool,
            tc.tile_pool(name="expp", bufs=34) as epool,
            tc.tile_pool(name="crp", bufs=3) as crpool,
            tc.tile_pool(name="ctp", bufs=2) as ctpool,
            tc.tile_pool(name="outp", bufs=4) as opool,
        ):
            # hot-path weights + first x chunks, in priority order: the K
            # projection of chunk 0 gates everything, so its DMAs go first
            # smallest possible first slices so the K chain's kc=0 matmul
            # fires as early as the DMA pipe allows
            # biases FIRST: the K/Q bias ACTs free the proj PSUM buffers,
            # and a late 1KB bias DMA was observed to stall the whole
            # Q-projection chain behind the PSUM pool rotation
            bq_sb = cpool.tile([128, 2], f32, name="bq_sb")
            nc.sync.dma_start(bq_sb[:], bq_d[:])
            bk_sb = cpool.tile([128, 2], f32, name="bk_sb")
            nc.sync.dma_start(bk_sb[:], bk_d[:])
            wk_sb = cpool.tile([128, KC, O], bf16, name="wk_sb")
            nc.sync.dma_start(wk_sb[:, 0:2, :], wkt_d[:, 0:2, :])
            nc.sync.dma_start(wk_sb[:, 2:8, :], wkt_d[:, 2:8, :])
            xk0 = xpool.tile([128, KC, QT_BLK], bf16, name="xk", tag="xk")
            nc.sync.dma_start(xk0[:, 0:1, :], xkt_d[0][:, 0:1, :])
            nc.sync.dma_start(xk0[:, 1:2, :], xkt_d[0][:, 1:2, :])
            nc.sync.dma_start(xk0[:, 2:4, :], xkt_d[0][:, 2:4, :])
            nc.sync.dma_start(xk0[:, 4:6, :], xkt_d[0][:, 4:6, :])
            nc.sync.dma_start(xk0[:, 6:8, :], xkt_d[0][:, 6:8, :])
            # fine slices: in-flight DMAs round-robin, so small slices land
            # early and the Q/V chains stream at kc granularity instead of
            # waiting for whole tensors
            wq_sb = cpool.tile([128, KC, O], bf16, name="wq_sb")
            nc.sync.dma_start(wq_sb[:, 0:2, :], wqt_d[:, 0:2, :])
            nc.sync.dma_start(wq_sb[:, 2:8, :], wqt_d[:, 2:8, :])
            xq0 = xpool.tile([128, KC, QT_BLK], bf16, name="xq", tag="xq")
            nc.sync.dma_start(xq0[:, 0:1, :], xqt_d[0][:, 0:1, :])
            nc.sync.dma_start(xq0[:, 1:2, :], xqt_d[0][:, 1:2, :])
            nc.sync.dma_start(xq0[:, 2:4, :], xqt_d[0][:, 2:4, :])
            nc.sync.dma_start(xq0[:, 4:8, :], xqt_d[0][:, 4:8, :])
            wv_sb = cpool.tile([128, KC, O], bf16, name="wv_sb")
            nc.sync.dma_start(wv_sb[:, 0:2, :], wvt_d[:, 0:2, :])
            nc.sync.dma_start(wv_sb[:, 2:8, :], wvt_d[:, 2:8, :])
            xv0 = xpool.tile([128, KC, QT_BLK], bf16, name="xv", tag="xv")
            nc.sync.dma_start(xv0[:, 0:1, :], xvt_d[0][:, 0:1, :])
            nc.sync.dma_start(xv0[:, 1:2, :], xvt_d[0][:, 1:2, :])
            nc.sync.dma_start(xv0[:, 2:4, :], xvt_d[0][:, 2:4, :])
            nc.sync.dma_start(xv0[:, 4:8, :], xvt_d[0][:, 4:8, :])
            bvb_sb = cpool.tile([128, O], f32, name="bvb_sb")
            nc.sync.dma_start(bvb_sb[:], bvb_d[:])

            qt_sb = cpool.tile([128, 2, S], bf16, name="qt_sb")
            kt_sb = cpool.tile([128, 2, S], bf16, name="kt_sb")
            # AV weights: per k-tile/pair, even head [V|ones] (den @ row 64),
            # odd head [ones|0|V] (den @ row 0, C^T @ rows 64-127)
            vaug_e = cpool.tile([128, 16, 2, 66], bf16, name="vaug_e")
            nc.vector.memset(vaug_e[:], 1.0)
            vaug_o = cpool.tile([128, 16, 2, 128], bf16, name="vaug_o")
            nc.vector.memset(vaug_o[:], 0.0)
            nc.vector.memset(vaug_o[:, :, :, 0:1], 1.0)
            # raw-denominator staging rows 0 (odd head) / 64 (even head);
            # rows 1-63 stay 1.0 (multiplied by bsel zeros in the bcast).
            # memset can't emit f32r, so memset f32 and cast-copy once.
            onesf = cpool.tile([65, QT_BLK], f32, name="onesf")
            nc.vector.memset(onesf[:], 1.0)
            dsb = cpool.tile([65, 2, QT_BLK], f32r, name="dsb")
            nc.vector.tensor_copy(dsb[:, 0, :], onesf[:])
            nc.vector.tensor_copy(dsb[:, 1, :], onesf[:])

            xq_t, xk_t, xv_t = {}, {}, {}

            def dma_chunk(sc, eng=None):
                eng = eng or nc.sync
                xk = xpool.tile([128, KC, QT_BLK], bf16, name="xk", tag="xk")
                eng.dma_start(xk[:, 0:4, :], xkt_d[sc][:, 0:4, :])
                eng.dma_start(xk[:, 4:8, :], xkt_d[sc][:, 4:8, :])
                xq = xpool.tile([128, KC, QT_BLK], bf16, name="xq", tag="xq")
                eng.dma_start(xq[:, 0:4, :], xqt_d[sc][:, 0:4, :])
                eng.dma_start(xq[:, 4:8, :], xqt_d[sc][:, 4:8, :])
                xv = xpool.tile([128, KC, QT_BLK], bf16, name="xv", tag="xv")
                eng.dma_start(xv[:, 0:4, :], xvt_d[sc][:, 0:4, :])
                eng.dma_start(xv[:, 4:8, :], xvt_d[sc][:, 4:8, :])
                xk_t[sc], xq_t[sc], xv_t[sc] = xk, xq, xv

            with tc.tile_pool(name="ps", bufs=2, space="PSUM") as ps:

                def proj_units(sc):
                    """Projection for chunk sc as a list of small emission
                    units (a few matmuls each) so they can be interleaved
                    into attention sweeps as PE filler."""
                    ssl = bass.ds(sc * QT_BLK, QT_BLK)
                    xk, xq, xv = xk_t[sc], xq_t[sc], xv_t[sc]
                    units = []

                    def chain(w_sb, x, b_sb, dst_tile, dst_hp, ot):
                        box = {}

                        def u1():
                            box["p"] = ps.tile([128, QT_BLK], f32, name="pp",
                                               tag="big", bufs=2)
                            for kc in range(4):
                                nc.tensor.matmul(
                                    box["p"][:],
                                    w_sb[:, kc, bass.ds(ot * 128, 128)],
                                    x[:, kc, :], start=(kc == 0), stop=False,
                                    skip_group_check=True)

                        def u2():
                            for kc in range(4, KC):
                                nc.tensor.matmul(
                                    box["p"][:],
                                    w_sb[:, kc, bass.ds(ot * 128, 128)],
                                    x[:, kc, :], start=False,
                                    stop=(kc == KC - 1),
                                    skip_group_check=True)
                            nc.scalar.activation(dst_tile[:, dst_hp, ssl],
                                                 box["p"][:], IDF,
                                                 bias=b_sb[:, ot:ot + 1])
                        units.extend([u1, u2])

                    for ot in range(2):
                        chain(wk_sb, xk, bk_sb, kt_sb, ot, ot)
                    for ot in range(2):
                        chain(wq_sb, xq, bq_sb, qt_sb, ot, ot)

                    def vchain(mtp):
                        box = {}

                        def u1():
                            box["p"] = ps.tile([128, QT_BLK], f32, name="pp",
                                               tag="big", bufs=2)
                            for kc in range(KC):
                                nc.tensor.matmul(
                                    box["p"][:, 0:O],
                                    xv[:, kc, bass.ds(2 * mtp * 128, 128)],
                                    wv_sb[:, kc, :], start=(kc == 0),
                                    stop=(kc == KC - 1),
                                    skip_group_check=True)

                        def u2():
                            for kc in range(KC):
                                nc.tensor.matmul(
                                    box["p"][:, O:2 * O],
                                    xv[:, kc,
                                       bass.ds((2 * mtp + 1) * 128, 128)],
                                    wv_sb[:, kc, :], start=(kc == 0),
                                    stop=(kc == KC - 1),
                                    skip_group_check=True)
                            pv = box["p"]
                            pvr = pv[:].rearrange(
                                "p (mt hp two d) -> p mt hp two d",
                                mt=2, hp=2, two=2)
                            bvr = bvb_sb[:].rearrange(
                                "p (hp two d) -> p hp two d", hp=2, two=2)
                            for mt2 in range(2):
                                kt = sc * 4 + 2 * mtp + mt2
                                nc.vector.tensor_tensor(
                                    vaug_e[:, kt, :, 0:64],
                                    pvr[:, mt2, :, 0, :], bvr[:, :, 0, :],
                                    mybir.AluOpType.add)
                                nc.vector.tensor_tensor(
                                    vaug_o[:, kt, :, 64:128],
                                    pvr[:, mt2, :, 1, :], bvr[:, :, 1, :],
                                    mybir.AluOpType.add)
                        units.extend([u1, u2])

                    for mtp in range(2):
                        vchain(mtp)
                    return units

                def proj(sc, mid=None):
                    units = proj_units(sc)
                    for i, u in enumerate(units):
                        if i == 4 and mid is not None:
                            mid()
                        u()

                xk_t[0], xq_t[0], xv_t[0] = xk0, xq0, xv0
                # chunk-1 + tail constants dispatch from the Scalar DGE,
                # queued behind proj(0)'s K-bias ACTs, so the in-flight DMA
                # set stays small while xq0/xv0 stream (packets of all
                # in-flight DMAs share the queues round-robin)
                consts = {}

                def _lead_mid():
                    dma_chunk(1)
                    bsel_sb = cpool.tile([65, 128], f32r, name="bsel_sb")
                    nc.sync.dma_start(bsel_sb[:], bsel_d[:])
                    wo_sb = cpool.tile([128, 2, D], bf16, name="wo_sb")
                    nc.sync.dma_start(wo_sb[:, 0, :], wot_d[:, 0, :])
                    nc.sync.dma_start(wo_sb[:, 1, :], wot_d[:, 1, :])
                    bo_sb = cpool.tile([128, 8], f32, name="bo_sb")
                    nc.sync.dma_start(bo_sb[:], bo_d[:])
                    consts.update(bsel_sb=bsel_sb, wo_sb=wo_sb, bo_sb=bo_sb)

                proj(0, mid=_lead_mid)
                bsel_sb = consts["bsel_sb"]
                wo_sb = consts["wo_sb"]
                bo_sb = consts["bo_sb"]

                ct_t = {}
                prev = None

                def bcast_norm(state):
                    # broadcast raw dens across partitions (PE), reciprocal
                    # in SBUF at partition base 0 (recip_approx_fast only
                    # works there), then normalize ctraw -> ct
                    qc, hp, ctraw, pp = state
                    pdup = ps.tile([128, QT_BLK], f32, name="pdup",
                                   tag="big", bufs=2)
                    nc.tensor.matmul(pdup[:], bsel_sb[:], dsb[0:65, pp, :],
                                     start=True, stop=True)
                    pbcs = crpool.tile([128, QT_BLK], f32, name="pbcs",
                                       tag="pbcs")
                    nc.vector.tensor_copy(pbcs[:], pdup[:])
                    pbcr = crpool.tile([128, QT_BLK], f32, name="pbcr",
                                       tag="pbcr")
                    nc.vector.reciprocal_approx_fast(pbcr[:], pbcs[:])
                    if hp == 0:
                        ct = ctpool.tile([128, 2, QT_BLK], bf16, name="ct",
                                         tag="ct")
                        ct_t[qc] = ct
                    ct = ct_t[qc]
                    nc.vector.tensor_tensor(ct[:, hp, :], ctraw[:, :],
                                            pbcr[:, :], mybir.AluOpType.mult)

                def wo_units(qc, jts=range(8), bias_split=False,
                             dma_split=1):
                    ct = ct_t[qc]
                    units = []

                    def mk(jt):
                        def u():
                            pwo = ps.tile([128, QT_BLK], f32, name="pwo",
                                          tag="big", bufs=2)
                            for kc in range(2):
                                nc.tensor.matmul(
                                    pwo[:],
                                    wo_sb[:, kc, bass.ds(jt * 128, 128)],
                                    ct[:, kc, :], start=(kc == 0),
                                    stop=(kc == 1), skip_group_check=True)
                            if bias_split and jt % 2:
                                # DVE path uses its own pool tag: one writer
                                # engine per tag (mixed writers on one tag
                                # proved race-prone on this runtime)
                                osb = opool.tile([128, QT_BLK], bf16,
                                                 name="osbv", tag="osbv",
                                                 bufs=2)
                                nc.vector.tensor_scalar(
                                    osb[:], pwo[:], bo_sb[:, jt:jt + 1],
                                    None, mybir.AluOpType.add)
                            else:
                                osb = opool.tile([128, QT_BLK], bf16,
                                                 name="osb", tag="osb")
                                nc.scalar.activation(osb[:], pwo[:], IDF,
                                                     bias=bo_sb[:, jt:jt + 1])
                            # a single 128KB dma rides ~one queue (~27GB/s);
                            # split the drain-phase stores across queues so
                            # the end-of-kernel barrier isn't stuck on them
                            w = QT_BLK // dma_split
                            for sp in range(dma_split):
                                qs = bass.ds(qc * QT_BLK + sp * w, w)
                                nc.sync.dma_start(out_d[jt][:, qs],
                                                  osb[:, bass.ds(sp * w, w)])
                        return u

                    for jt in jts:
                        units.append(mk(jt))
                    return units

                def wo(qc, jts=range(8), bias_split=False, dma_split=1):
                    for u in wo_units(qc, jts, bias_split, dma_split):
                        u()

                # Sweep-level software pipeline: sweep N emits only QK+exp
                # (+affine); its AV matmuls are interleaved into sweep N+1's
                # k-tile loop.  This keeps the PE stream dense (long streaks
                # ramp the PE p-state) and gives the slow gpsimd affine hop a
                # whole sweep of slack before its AV consumer.
                def emit_av(s, idx):
                    ki, et, lo = s["ets"][idx]
                    alo = 0 if idx == 0 else lo
                    last = s["n_ki"] - 1
                    nc.tensor.matmul(
                        s["pave"][:, alo:QT_BLK],
                        vaug_e[:, ki, s["hp"], 0:65],
                        et[:, 0, alo:QT_BLK],
                        start=(idx == 0), stop=(idx == last),
                        skip_group_check=True)
                    nc.tensor.matmul(
                        s["pavo"][:, alo:QT_BLK],
                        vaug_o[:, ki, s["hp"], :],
                        et[:, 1, alo:QT_BLK],
                        start=(idx == 0), stop=(idx == last),
                        skip_group_check=True)

                def dens(s):
                    # raw denominators first (they gate the pdup broadcast),
                    # then raw C^T out of PSUM (frees pav)
                    pp = s["hp"]
                    nc.vector.tensor_copy(dsb[64:65, pp, :],
                                          s["pave"][64:65, :])
                    nc.vector.tensor_copy(dsb[0:1, pp, :],
                                          s["pavo"][0:1, :])
                    ctraw = crpool.tile([128, QT_BLK], bf16,
                                        name="ctraw", tag="ctraw")
                    nc.vector.tensor_copy(ctraw[0:64, :], s["pave"][0:64, :])
                    nc.vector.tensor_copy(ctraw[64:128, :],
                                          s["pavo"][64:128, :])
                    return (s["qc"], s["hp"], ctraw, pp)

                prev_sw = None   # sweep whose AVs run during current sweep
                pend_norm = None  # dens() output awaiting bcast_norm
                wo_ready = []    # qc values whose ct is fully normalized

                for qc in range(N_QC):
                    for hp in range(2):
                        n_ki = 4 * qc + 4
                        if prev_sw is not None:
                            prev_sw["pave"] = ps.tile([65, QT_BLK], f32,
                                                      name="pav_e",
                                                      tag="pave", bufs=1)
                            prev_sw["pavo"] = ps.tile([128, QT_BLK], f32,
                                                      name="pav_o",
                                                      tag="pavo", bufs=1)
                        n_av = len(prev_sw["ets"]) if prev_sw else 0
                        av_done = 0
                        # PE filler for Scalar-paced hp=0 sweeps: the next
                        # chunk's projections and the pending Wo, emitted in
                        # small units between k-tiles (their input data
                        # landed sweeps ago)
                        fill = []
                        if hp == 0 and qc >= 2:
                            if qc < 3:
                                fill += proj_units(qc + 1)
                            if wo_ready:
                                # bias_split: half the bias ops go to DVE so
                                # they don't sit in the Scalar stream ahead
                                # of this sweep's exps (Scalar paces hp=0
                                # sweeps)
                                fill += wo_units(wo_ready.pop(0),
                                                 bias_split=True)
                        f_done = 0
                        # last sweep: front-load the previous sweep's AVs,
                        # free its pav mid-sweep, then run our own AVs
                        # inline so the flush only drains a couple of tiles
                        # (tried: inline last-sweep AVs; regressed 27us -
                        # front-loaded AVs starve the st/exp pipeline)
                        special = False
                        sp_state = None
                        sp_norm = None
                        sp_done = 0
                        ets = []
                        for ki in range(n_ki):
                            lo = max(0, 128 * ki - QT_BLK * qc)
                            st = ps.tile([128, 2, QT_BLK], f32, name="st",
                                         tag="st", bufs=2)
                            for side in range(2):
                                po = bass.ds(side * 64, 64)
                                nc.tensor.matmul(
                                    st[:, side, lo:QT_BLK],
                                    kt_sb[po, hp, bass.ds(ki * 128, 128)],
                                    qt_sb[po, hp,
                                          bass.ds(qc * QT_BLK + lo,
                                                  QT_BLK - lo)],
                                    start=True, stop=True)
                            et = epool.tile([128, 2, QT_BLK], bf16,
                                            name="et", tag="et")
                            nc.scalar.activation(et[:, :, lo:QT_BLK],
                                                 st[:, :, lo:QT_BLK], EXPF,
                                                 scale=0.125)
                            if ki >= 4 * qc:
                                # zero the causal staircase (cols lo..lo+127)
                                nc.gpsimd.affine_select(
                                    out=et[:, :, bass.ds(lo, 128)],
                                    in_=et[:, :, bass.ds(lo, 128)],
                                    compare_op=mybir.AluOpType.is_ge,
                                    fill=0.0, base=0,
                                    pattern=[[0, 2], [1, 128]],
                                    channel_multiplier=-1)
                            ets.append((ki, et, lo))
                            if special:
                                want = min(n_av, 2 * (ki + 1))
                            else:
                                want = (n_av * (ki + 1)) // n_ki
                            while av_done < want:
                                emit_av(prev_sw, av_done)
                                av_done += 1
                            if special and av_done == n_av and ki >= 10:
                                if sp_state is None:
                                    sp_norm = dens(prev_sw)
                                    sp_state = {
                                        "qc": qc, "hp": hp, "n_ki": n_ki,
                                        "ets": ets,
                                        "pave": ps.tile([65, QT_BLK], f32,
                                                        name="pav_e",
                                                        tag="pave", bufs=1),
                                        "pavo": ps.tile([128, QT_BLK], f32,
                                                        name="pav_o",
                                                        tag="pavo", bufs=1),
                                    }
                                while sp_done < max(0, len(ets) - 2):
                                    emit_av(sp_state, sp_done)
                                    sp_done += 1
                            want_f = (len(fill) * (ki + 1)) // n_ki
                            while f_done < want_f:
                                fill[f_done]()
                                f_done += 1
                        while av_done < n_av:
                            emit_av(prev_sw, av_done)
                            av_done += 1
                        while f_done < len(fill):
                            fill[f_done]()
                            f_done += 1
                        if special:
                            new_norm = sp_norm
                        else:
                            new_norm = dens(prev_sw) if prev_sw else None
                        if pend_norm is not None:
                            bcast_norm(pend_norm)
                            if pend_norm[1] == 1:
                                wo_ready.append(pend_norm[0])
                        pend_norm = new_norm
                        if hp == 0:
                            if qc + 2 < N_QC:
                                dma_chunk(qc + 2)
                            if qc < 2:
                                proj(qc + 1)
                        if special:
                            sp_state["done"] = sp_done
                            prev_sw = sp_state
                        else:
                            prev_sw = {"qc": qc, "hp": hp, "n_ki": n_ki,
                                       "ets": ets}

                # flush: remaining AVs of the last sweep + trailing norms/Wo
                if "pave" not in prev_sw:
                    prev_sw["pave"] = ps.tile([65, QT_BLK], f32,
                                              name="pav_e", tag="pave",
                                              bufs=1)
                    prev_sw["pavo"] = ps.tile([128, QT_BLK], f32,
                                              name="pav_o", tag="pavo",
                                              bufs=1)
                for idx in range(prev_sw.get("done", 0),
                                 len(prev_sw["ets"])):
                    emit_av(prev_sw, idx)
                last_norm = dens(prev_sw)
                bcast_norm(pend_norm)  # (3, 0)
                # wo(2) split around the final normalize so its matmuls fill
                # the PE while the (3,1) reciprocal chain runs on DVE
                # (dma_split > 1 regressed: column-sliced stores drop the
                # per-partition contiguous run to 256B, halving DMA
                # efficiency - keep whole-tile stores)
                wo(2, jts=range(0, 4))
                bcast_norm(last_norm)  # (3, 1)
                wo(2, jts=range(4, 8), bias_split=True)
                wo(3, bias_split=True)

    nc.compile()
    return nc


def kernel(query, key, value, mask, Wq, bq, Wk, bk, Wv, bv, Wo, bo):
    query = np.asarray(query, np.float32)
    key_ = np.asarray(key, np.float32)
    value = np.asarray(value, np.float32)
    Wq, Wk, Wv, Wo = (np.asarray(w, np.float32) for w in (Wq, Wk, Wv, Wo))
    bq, bk, bv, bo = (np.asarray(b_, np.float32) for b_ in (bq, bk, bv, bo))

    mask = np.asarray(mask)
    assert np.array_equal(mask != 0, np.tril(np.ones((S, S), bool))), \
        "kernel is specialized to the causal mask"
    if "nc" not in _CACHE:
        _CACHE["nc"] = _build()
    nc = _CACHE["nc"]

    def xt(x):  # [S, D] -> [N_QC, 128, KC, QT_BLK] bf16, partition-major
        a = x.T.reshape(KC, 128, S).transpose(1, 0, 2)  # [128, KC, S]
        a = a.reshape(128, KC, N_QC, QT_BLK).transpose(2, 0, 1, 3)
        return np.ascontiguousarray(a).astype(BF16)

    def wslice(W, c):  # [D, D] -> [128, KC, O] bf16 of W[o_slice].T
        hg = c % TP
        a = W[hg * O:(hg + 1) * O].T.reshape(KC, 128, O).transpose(1, 0, 2)
        return np.ascontiguousarray(a).astype(BF16)

    # bcast selector: rows 0-63 of pbc take 1/den_even (rdsb row 64),
    # rows 64-127 take 1/den_odd (rdsb row 0)
    bsel = np.zeros((65, 128), np.float32)
    bsel[64, 0:64] = 1.0
    bsel[0, 64:128] = 1.0

    in_maps = []
    for c in range(N_CORES):
        b_, hg = c // TP, c % TP
        osl = slice(hg * O, (hg + 1) * O)
        bo_part = bo if hg == 0 else np.zeros_like(bo)
        wot = Wo[:, osl].T.reshape(2, 128, D).transpose(1, 0, 2)
        in_maps.append({
            "xqt": xt(query[b_]),
            "xkt": xt(key_[b_]),
            "xvt": xt(value[b_]),
            "wqt": wslice(Wq, c),
            "wkt": wslice(Wk, c),
            "wvt": wslice(Wv, c),
            "wot": np.ascontiguousarray(wot).astype(BF16),
            "bqc": np.ascontiguousarray(bq[osl].reshape(2, 128).T),
            "bkc": np.ascontiguousarray(bk[osl].reshape(2, 128).T),
            "bvb": np.ascontiguousarray(np.broadcast_to(bv[osl], (128, O))),
            "boc": np.ascontiguousarray(bo_part.reshape(8, 128).T),
            "bsel": bsel,
        })

    res = run_bass_kernel_spmd(nc, in_maps, core_ids=list(range(N_CORES)))

    out = np.zeros((B, S, D), np.float32)
    for c in range(N_CORES):
        part = res.results[c]["out"].reshape(D, S)  # out^T [j, s]
        out[c // TP] += part.T.astype(np.float32)
    return out


# revision 69
# speedup vs baseline: 1.2349x; 1.0108x over previous
"""Trainium2 Bass kernel for nn_MultiHeadAttention_76295799046818.

MHA: B=2, S=2048, D=1024, H=16 heads (d_k=64), causal, fp32 reference.
Sharded over 8 NeuronCores: data-parallel over batch (2) x tensor-parallel
over heads (4 heads/core).  Wq/Wk/Wv column-parallel; Wo row-parallel with
the 4 partial outputs per batch summed on the host (cheaper than an
on-device all-reduce in this runtime).

Single fused pipeline per core (bf16 matmuls, fp32 PSUM):
  - Projections are chunked by 512 rows of S and interleaved with attention:
    chunk sc feeds attention q-chunk qc=sc, so QK/exp start ~10us in while
    later chunks still stream from HBM.
  - Per (head-pair hp, q-chunk qc) sweep: QK + exp per k-tile, with the
    sweep's AV matmuls DEFERRED one sweep and interleaved into the next
    sweep's k-tile loop (keeps the PE stream dense and gives the slow
    gpsimd semaphore hop a whole sweep of slack).  The pair's two heads
    sit at SBUF partitions 0-63/64-127 so their K=64 QK matmuls run on
    disjoint PE row groups concurrently.  exp is sliced to live columns;
    the causal staircase inside diagonal tiles is zeroed by gpsimd
    affine_select on the bf16 exp tile (no PE mask matmuls, no mask DMA).
  - AV accumulates [V|ones] / [ones|0|V] weights so softmax denominators
    land at PSUM rows 64 (even head) / 0 (odd head); C^T raw is copied out
    bf16 immediately (frees PSUM); the raw denominators are broadcast
    across partitions by a one-hot PE matmul, reciprocal'd with DVE
    reciprocal_approx_fast (only works on partition-base-0 SBUF APs), and
    a DVE multiply normalizes - trailing the sweep by one more slot.
  - Wo partials per q-chunk trail two slots behind, interleaved as PE
    filler units (with next-chunk projections) into the Scalar-paced
    hp=0 sweeps; output bias via Scalar ACT, bf16 DMA out.
"""

import numpy as np
import ml_dtypes

import concourse.bass as bass
import concourse.mybir as mybir
import concourse.tile as tile
from concourse import bacc
from concourse.bass_utils import run_bass_kernel_spmd

BF16 = ml_dtypes.bfloat16

B, S, D, H, DK = 2, 2048, 1024, 16, 64
N_CORES = 8
TP = 4  # head-parallel degree (per batch)
HPC = H // TP  # heads per core = 4
O = HPC * DK  # output channels per core = 256
QT_BLK = 512
N_QC = S // QT_BLK  # 4
KC = D // 128  # 8 contraction chunks for projections

_CACHE = {}


def _build():
    nc = bacc.Bacc("TRN2", target_bir_lowering=False, debug=False,
                   num_devices=N_CORES)
    dt = mybir.dt
    f32, bf16, f32r = dt.float32, dt.bfloat16, dt.float32r

    def din(name, shape, dtype=bf16):
        return nc.dram_tensor(name, shape, dtype, kind="ExternalInput").ap()

    xqt_d = din("xqt", [N_QC, 128, KC, QT_BLK])
    xkt_d = din("xkt", [N_QC, 128, KC, QT_BLK])
    xvt_d = din("xvt", [N_QC, 128, KC, QT_BLK])
    wqt_d = din("wqt", [128, KC, O])
    wkt_d = din("wkt", [128, KC, O])
    wvt_d = din("wvt", [128, KC, O])
    wot_d = din("wot", [128, 2, D])
    bq_d = din("bqc", [128, 2], f32)
    bk_d = din("bkc", [128, 2], f32)
    bvb_d = din("bvb", [128, O], f32)
    bo_d = din("boc", [128, 8], f32)
    bsel_d = din("bsel", [65, 128], f32r)
    out_d = nc.dram_tensor("out", [8, 128, S], bf16,
                           kind="ExternalOutput").ap()

    EXPF = mybir.ActivationFunctionType.Exp
    IDF = mybir.ActivationFunctionType.Identity

    with tile.TileContext(nc) as tc:
        with (
            tc.tile_pool(name="const", bufs=1) as cpool,
            tc.tile_pool(name="xin", bufs=2) as xpool,
            tc.tile_pool(name="expp", bufs=34) as epool,
            tc.tile_pool(name="crp", bufs=3) as crpool,
            tc.tile_pool(name="ctp", bufs=2) as ctpool,
            tc.tile_pool(name="outp", bufs=4) as opool,
        ):
            # hot-path weights + first x chunks, in priority order: the K
            # projection of chunk 0 gates everything, so its DMAs go first
            # smallest possible first slices so the K chain's kc=0 matmul
            # fires as early as the DMA pipe allows
            # biases FIRST: the K/Q bias ACTs free the proj PSUM buffers,
            # and a late 1KB bias DMA was observed to stall the whole
            # Q-projection chain behind the PSUM pool rotation
            bq_sb = cpool.tile([128, 2], f32, name="bq_sb")
            nc.sync.dma_start(bq_sb[:], bq_d[:])
            bk_sb = cpool.tile([128, 2], f32, name="bk_sb")
            nc.sync.dma_start(bk_sb[:], bk_d[:])
            wk_sb = cpool.tile([128, KC, O], bf16, name="wk_sb")
            nc.sync.dma_start(wk_sb[:, 0:2, :], wkt_d[:, 0:2, :])
            nc.sync.dma_start(wk_sb[:, 2:8, :], wkt_d[:, 2:8, :])
            xk0 = xpool.tile([128, KC, QT_BLK], bf16, name="xk", tag="xk")
            nc.sync.dma_start(xk0[:, 0:1, :], xkt_d[0][:, 0:1, :])
            nc.sync.dma_start(xk0[:, 1:2, :], xkt_d[0][:, 1:2, :])
            nc.sync.dma_start(xk0[:, 2:4, :], xkt_d[0][:, 2:4, :])
            nc.sync.dma_start(xk0[:, 4:6, :], xkt_d[0][:, 4:6, :])
            nc.sync.dma_start(xk0[:, 6:8, :], xkt_d[0][:, 6:8, :])
            # fine slices: in-flight DMAs round-robin, so small slices land
            # early and the Q/V chains stream at kc granularity instead of
            # waiting for whole tensors
            wq_sb = cpool.tile([128, KC, O], bf16, name="wq_sb")
            nc.sync.dma_start(wq_sb[:, 0:2, :], wqt_d[:, 0:2, :])
            nc.sync.dma_start(wq_sb[:, 2:8, :], wqt_d[:, 2:8, :])
            xq0 = xpool.tile([128, KC, QT_BLK], bf16, name="xq", tag="xq")
            nc.sync.dma_start(xq0[:, 0:1, :], xqt_d[0][:, 0:1, :])
            nc.sync.dma_start(xq0[:, 1:2, :], xqt_d[0][:, 1:2, :])
            nc.sync.dma_start(xq0[:, 2:4, :], xqt_d[0][:, 2:4, :])
            nc.sync.dma_start(xq0[:, 4:8, :], xqt_d[0][:, 4:8, :])
            wv_sb = cpool.tile([128, KC, O], bf16, name="wv_sb")
            nc.sync.dma_start(wv_sb[:, 0:2, :], wvt_d[:, 0:2, :])
            nc.sync.dma_start(wv_sb[:, 2:8, :], wvt_d[:, 2:8, :])
            xv0 = xpool.tile([128, KC, QT_BLK], bf16, name="xv", tag="xv")
            nc.sync.dma_start(xv0[:, 0:1, :], xvt_d[0][:, 0:1, :])
            nc.sync.dma_start(xv0[:, 1:2, :], xvt_d[0][:, 1:2, :])
            nc.sync.dma_start(xv0[:, 2:4, :], xvt_d[0][:, 2:4, :])
            nc.sync.dma_start(xv0[:, 4:8, :], xvt_d[0][:, 4:8, :])
            bvb_sb = cpool.tile([128, O], f32, name="bvb_sb")
            nc.sync.dma_start(bvb_sb[:], bvb_d[:])

            qt_sb = cpool.tile([128, 2, S], bf16, name="qt_sb")
            kt_sb = cpool.tile([128, 2, S], bf16, name="kt_sb")
            # AV weights: per k-tile/pair, even head [V|ones] (den @ row 64),
            # odd head [ones|0|V] (den @ row 0, C^T @ rows 64-127)
            vaug_e = cpool.tile([128, 16, 2, 66], bf16, name="vaug_e")
            nc.vector.memset(vaug_e[:], 1.0)
            vaug_o = cpool.tile([128, 16, 2, 128], bf16, name="vaug_o")
            nc.vector.memset(vaug_o[:], 0.0)
            nc.vector.memset(vaug_o[:, :, :, 0:1], 1.0)
            # raw-denominator staging rows 0 (odd head) / 64 (even head);
            # rows 1-63 stay 1.0 (multiplied by bsel zeros in the bcast).
            # memset can't emit f32r, so memset f32 and cast-copy once.
            onesf = cpool.tile([65, QT_BLK], f32, name="onesf")
            nc.vector.memset(onesf[:], 1.0)
            dsb = cpool.tile([65, 2, QT_BLK], f32r, name="dsb")
            nc.vector.tensor_copy(dsb[:, 0, :], onesf[:])
            nc.vector.tensor_copy(dsb[:, 1, :], onesf[:])

            xq_t, xk_t, xv_t = {}, {}, {}

            def dma_chunk(sc, eng=None):
                eng = eng or nc.sync
                xk = xpool.tile([128, KC, QT_BLK], bf16, name="xk", tag="xk")
                eng.dma_start(xk[:, 0:4, :], xkt_d[sc][:, 0:4, :])
                eng.dma_start(xk[:, 4:8, :], xkt_d[sc][:, 4:8, :])
                xq = xpool.tile([128, KC, QT_BLK], bf16, name="xq", tag="xq")
                eng.dma_start(xq[:, 0:4, :], xqt_d[sc][:, 0:4, :])
                eng.dma_start(xq[:, 4:8, :], xqt_d[sc][:, 4:8, :])
                xv = xpool.tile([128, KC, QT_BLK], bf16, name="xv", tag="xv")
                eng.dma_start(xv[:, 0:4, :], xvt_d[sc][:, 0:4, :])
                eng.dma_start(xv[:, 4:8, :], xvt_d[sc][:, 4:8, :])
                xk_t[sc], xq_t[sc], xv_t[sc] = xk, xq, xv

            with tc.tile_pool(name="ps", bufs=2, space="PSUM") as ps:

                def proj_units(sc):
                    """Projection for chunk sc as a list of small emission
                    units (a few matmuls each) so they can be interleaved
                    into attention sweeps as PE filler."""
                    ssl = bass.ds(sc * QT_BLK, QT_BLK)
                    xk, xq, xv = xk_t[sc], xq_t[sc], xv_t[sc]
                    units = []

                    def chain(w_sb, x, b_sb, dst_tile, dst_hp, ot):
                        box = {}

                        def u1():
                            box["p"] = ps.tile([128, QT_BLK], f32, name="pp",
                                               tag="big", bufs=2)
                            for kc in range(4):
                                nc.tensor.matmul(
                                    box["p"][:],
                                    w_sb[:, kc, bass.ds(ot * 128, 128)],
                                    x[:, kc, :], start=(kc == 0), stop=False,
                                    skip_group_check=True)

                        def u2():
                            for kc in range(4, KC):
                                nc.tensor.matmul(
                                    box["p"][:],
                                    w_sb[:, kc, bass.ds(ot * 128, 128)],
                                    x[:, kc, :], start=False,
                                    stop=(kc == KC - 1),
                                    skip_group_check=True)
                            nc.scalar.activation(dst_tile[:, dst_hp, ssl],
                                                 box["p"][:], IDF,
                                                 bias=b_sb[:, ot:ot + 1])
                        units.extend([u1, u2])

                    for ot in range(2):
                        chain(wk_sb, xk, bk_sb, kt_sb, ot, ot)
                    for ot in range(2):
                        chain(wq_sb, xq, bq_sb, qt_sb, ot, ot)

                    def vchain(mtp):
                        box = {}

                        def u1():
                            box["p"] = ps.tile([128, QT_BLK], f32, name="pp",
                                               tag="big", bufs=2)
                            for kc in range(KC):
                                nc.tensor.matmul(
                                    box["p"][:, 0:O],
                                    xv[:, kc, bass.ds(2 * mtp * 128, 128)],
                                    wv_sb[:, kc, :], start=(kc == 0),
                                    stop=(kc == KC - 1),
                                    skip_group_check=True)

                        def u2():
                            for kc in range(KC):
                                nc.tensor.matmul(
                                    box["p"][:, O:2 * O],
                                    xv[:, kc,
                                       bass.ds((2 * mtp + 1) * 128, 128)],
                                    wv_sb[:, kc, :], start=(kc == 0),
                                    stop=(kc == KC - 1),
                                    skip_group_check=True)
                            pv = box["p"]
                            pvr = pv[:].rearrange(
                                "p (mt hp two d) -> p mt hp two d",
                                mt=2, hp=2, two=2)
                            bvr = bvb_sb[:].rearrange(
                                "p (hp two d) -> p hp two d", hp=2, two=2)
                            for mt2 in range(2):
                                kt = sc * 4 + 2 * mtp + mt2
                                nc.vector.tensor_tensor(
                                    vaug_e[:, kt, :, 0:64],
                                    pvr[:, mt2, :, 0, :], bvr[:, :, 0, :],
                                    mybir.AluOpType.add)
                                nc.vector.tensor_tensor(
                                    vaug_o[:, kt, :, 64:128],
                                    pvr[:, mt2, :, 1, :], bvr[:, :, 1, :],
                                    mybir.AluOpType.add)
                        units.extend([u1, u2])

                    for mtp in range(2):
                        vchain(mtp)
                    return units

                def proj(sc, mid=None):
                    units = proj_units(sc)
                    for i, u in enumerate(units):
                        if i == 4 and mid is not None:
                            mid()
                        u()

                xk_t[0], xq_t[0], xv_t[0] = xk0, xq0, xv0
                # chunk-1 + tail constants dispatch from the Scalar DGE,
                # queued behind proj(0)'s K-bias ACTs, so the in-flight DMA
                # set stays small while xq0/xv0 stream (packets of all
                # in-flight DMAs share the queues round-robin)
                consts = {}

                def _lead_mid():
                    dma_chunk(1)
                    bsel_sb = cpool.tile([65, 128], f32r, name="bsel_sb")
                    nc.sync.dma_start(bsel_sb[:], bsel_d[:])
                    wo_sb = cpool.tile([128, 2, D], bf16, name="wo_sb")
                    nc.sync.dma_start(wo_sb[:, 0, :], wot_d[:, 0, :])
                    nc.sync.dma_start(wo_sb[:, 1, :], wot_d[:, 1, :])
                    bo_sb = cpool.tile([128, 8], f32, name="bo_sb")
                    nc.sync.dma_start(bo_sb[:], bo_d[:])
                    consts.update(bsel_sb=bsel_sb, wo_sb=wo_sb, bo_sb=bo_sb)

                proj(0, mid=_lead_mid)
                bsel_sb = consts["bsel_sb"]
                wo_sb = consts["wo_sb"]
                bo_sb = consts["bo_sb"]

                ct_t = {}
                prev = None

                def bcast_norm(state):
                    # broadcast raw dens across partitions (PE), reciprocal
                    # in SBUF at partition base 0 (recip_approx_fast only
                    # works there), then normalize ctraw -> ct
                    qc, hp, ctraw, pp = state
                    pdup = ps.tile([128, QT_BLK], f32, name="pdup",
                                   tag="big", bufs=2)
                    nc.tensor.matmul(pdup[:], bsel_sb[:], dsb[0:65, pp, :],
                                     start=True, stop=True)
                    pbcs = crpool.tile([128, QT_BLK], f32, name="pbcs",
                                       tag="pbcs")
                    nc.vector.tensor_copy(pbcs[:], pdup[:])
                    pbcr = crpool.tile([128, QT_BLK], f32, name="pbcr",
                                       tag="pbcr")
                    nc.vector.reciprocal_approx_fast(pbcr[:], pbcs[:])
                    if hp == 0:
                        ct = ctpool.tile([128, 2, QT_BLK], bf16, name="ct",
                                         tag="ct")
                        ct_t[qc] = ct
                    ct = ct_t[qc]
                    nc.vector.tensor_tensor(ct[:, hp, :], ctraw[:, :],
                                            pbcr[:, :], mybir.AluOpType.mult)

                def wo_units(qc, jts=range(8), bias_split=False,
                             dma_split=1):
                    ct = ct_t[qc]
                    units = []

                    def mk(jt):
                        def u():
                            pwo = ps.tile([128, QT_BLK], f32, name="pwo",
                                          tag="big", bufs=2)
                            for kc in range(2):
                                nc.tensor.matmul(
                                    pwo[:],
                                    wo_sb[:, kc, bass.ds(jt * 128, 128)],
                                    ct[:, kc, :], start=(kc == 0),
                                    stop=(kc == 1), skip_group_check=True)
                            if bias_split and jt % 2:
                                # DVE path uses its own pool tag: one writer
                                # engine per tag (mixed writers on one tag
                                # proved race-prone on this runtime)
                                osb = opool.tile([128, QT_BLK], bf16,
                                                 name="osbv", tag="osbv",
                                                 bufs=2)
                                nc.vector.tensor_scalar(
                                    osb[:], pwo[:], bo_sb[:, jt:jt + 1],
                                    None, mybir.AluOpType.add)
                            else:
                                osb = opool.tile([128, QT_BLK], bf16,
                                                 name="osb", tag="osb")
                                nc.scalar.activation(osb[:], pwo[:], IDF,
                                                     bias=bo_sb[:, jt:jt + 1])
                            # a single 128KB dma rides ~one queue (~27GB/s);
                            # split the drain-phase stores across queues so
                            # the end-of-kernel barrier isn't stuck on them
                            w = QT_BLK // dma_split
                            for sp in range(dma_split):
                                qs = bass.ds(qc * QT_BLK + sp * w, w)
                                nc.sync.dma_start(out_d[jt][:, qs],
                                                  osb[:, bass.ds(sp * w, w)])
                        return u

                    for jt in jts:
                        units.append(mk(jt))
                    return units

                def wo(qc, jts=range(8), bias_split=False, dma_split=1):
                    for u in wo_units(qc, jts, bias_split, dma_split):
                        u()

                # Sweep-level software pipeline: sweep N emits only QK+exp
                # (+affine); its AV matmuls are interleaved into sweep N+1's
                # k-tile loop.  This keeps the PE stream dense (long streaks
                # ramp the PE p-state) and gives the slow gpsimd affine hop a
                # whole sweep of slack before its AV consumer.
                def emit_av(s, idx):
                    ki, et, lo = s["ets"][idx]
                    alo = 0 if idx == 0 else lo
                    last = s["n_ki"] - 1
                    nc.tensor.matmul(
                        s["pave"][:, alo:QT_BLK],
                        vaug_e[:, ki, s["hp"], 0:65],
                        et[:, 0, alo:QT_BLK],
                        start=(idx == 0), stop=(idx == last),
                        skip_group_check=True)
                    nc.tensor.matmul(
                        s["pavo"][:, alo:QT_BLK],
                        vaug_o[:, ki, s["hp"], :],
                        et[:, 1, alo:QT_BLK],
                        start=(idx == 0), stop=(idx == last),
                        skip_group_check=True)

                def dens(s):
                    # raw denominators first (they gate the pdup broadcast),
                    # then raw C^T out of PSUM (frees pav)
                    pp = s["hp"]
                    nc.vector.tensor_copy(dsb[64:65, pp, :],
                                          s["pave"][64:65, :])
                    nc.vector.tensor_copy(dsb[0:1, pp, :],
                                          s["pavo"][0:1, :])
                    ctraw = crpool.tile([128, QT_BLK], bf16,
                                        name="ctraw", tag="ctraw")
                    nc.vector.tensor_copy(ctraw[0:64, :], s["pave"][0:64, :])
                    nc.vector.tensor_copy(ctraw[64:128, :],
                                          s["pavo"][64:128, :])
                    return (s["qc"], s["hp"], ctraw, pp)

                prev_sw = None   # sweep whose AVs run during current sweep
                pend_norm = None  # dens() output awaiting bcast_norm
                wo_ready = []    # qc values whose ct is fully normalized

                for qc in range(N_QC):
                    for hp in range(2):
                        n_ki = 4 * qc + 4
                        if prev_sw is not None:
                            prev_sw["pave"] = ps.tile([65, QT_BLK], f32,
                                                      name="pav_e",
                                                      tag="pave", bufs=1)
                            prev_sw["pavo"] = ps.tile([128, QT_BLK], f32,
                                                      name="pav_o",
                                                      tag="pavo", bufs=1)
                        n_av = len(prev_sw["ets"]) if prev_sw else 0
                        av_done = 0
                        # PE filler for Scalar-paced hp=0 sweeps: the next
                        # chunk's projections and the pending Wo, emitted in
                        # small units between k-tiles (their input data
                        # landed sweeps ago)
                        fill = []
                        fill_from = 0
                        if hp == 0 and qc >= 1:
                            if qc < 3:
                                fill += proj_units(qc + 1)
                            if qc == 1:
                                # chunk-2 input lands mid-sweep: only fill
                                # the tail k-tiles to avoid head-blocking
                                fill_from = n_ki // 2
                            if wo_ready:
                                # bias_split: half the bias ops go to DVE so
                                # they don't sit in the Scalar stream ahead
                                # of this sweep's exps (Scalar paces hp=0
                                # sweeps)
                                fill += wo_units(wo_ready.pop(0),
                                                 bias_split=True)
                        f_done = 0
                        # last sweep: front-load the previous sweep's AVs,
                        # free its pav mid-sweep, then run our own AVs
                        # inline so the flush only drains a couple of tiles
                        # (tried: inline last-sweep AVs; regressed 27us -
                        # front-loaded AVs starve the st/exp pipeline)
                        special = False
                        sp_state = None
                        sp_norm = None
                        sp_done = 0
                        ets = []
                        for ki in range(n_ki):
                            lo = max(0, 128 * ki - QT_BLK * qc)
                            st = ps.tile([128, 2, QT_BLK], f32, name="st",
                                         tag="st", bufs=2)
                            for side in range(2):
                                po = bass.ds(side * 64, 64)
                                nc.tensor.matmul(
                                    st[:, side, lo:QT_BLK],
                                    kt_sb[po, hp, bass.ds(ki * 128, 128)],
                                    qt_sb[po, hp,
                                          bass.ds(qc * QT_BLK + lo,
                                                  QT_BLK - lo)],
                                    start=True, stop=True)
                            et = epool.tile([128, 2, QT_BLK], bf16,
                                            name="et", tag="et")
                            nc.scalar.activation(et[:, :, lo:QT_BLK],
                                                 st[:, :, lo:QT_BLK], EXPF,
                                                 scale=0.125)
                            if ki >= 4 * qc:
                                # zero the causal staircase (cols lo..lo+127)
                                nc.gpsimd.affine_select(
                                    out=et[:, :, bass.ds(lo, 128)],
                                    in_=et[:, :, bass.ds(lo, 128)],
                                    compare_op=mybir.AluOpType.is_ge,
                                    fill=0.0, base=0,
                                    pattern=[[0, 2], [1, 128]],
                                    channel_multiplier=-1)
                            ets.append((ki, et, lo))
                            if special:
                                want = min(n_av, 2 * (ki + 1))
                            else:
                                want = (n_av * (ki + 1)) // n_ki
                            while av_done < want:
                                emit_av(prev_sw, av_done)
                                av_done += 1
                            if special and av_done == n_av and ki >= 10:
                                if sp_state is None:
                                    sp_norm = dens(prev_sw)
                                    sp_state = {
                                        "qc": qc, "hp": hp, "n_ki": n_ki,
                                        "ets": ets,
                                        "pave": ps.tile([65, QT_BLK], f32,
                                                        name="pav_e",
                                                        tag="pave", bufs=1),
                                        "pavo": ps.tile([128, QT_BLK], f32,
                                                        name="pav_o",
                                                        tag="pavo", bufs=1),
                                    }
                                while sp_done < max(0, len(ets) - 2):
                                    emit_av(sp_state, sp_done)
                                    sp_done += 1
                            if ki >= fill_from:
                                want_f = (len(fill) * (ki + 1 - fill_from)
                                          ) // (n_ki - fill_from)
                                while f_done < want_f:
                                    fill[f_done]()
                                    f_done += 1
                        while av_done < n_av:
                            emit_av(prev_sw, av_done)
                            av_done += 1
                        while f_done < len(fill):
                            fill[f_done]()
                            f_done += 1
                        if special:
                            new_norm = sp_norm
                        else:
                            new_norm = dens(prev_sw) if prev_sw else None
                        if pend_norm is not None:
                            bcast_norm(pend_norm)
                            if pend_norm[1] == 1:
                                wo_ready.append(pend_norm[0])
                        pend_norm = new_norm
                        if hp == 0:
                            if qc + 2 < N_QC:
                                dma_chunk(qc + 2)
                            if qc < 1:
                                proj(qc + 1)
                        if special:
                            sp_state["done"] = sp_done
                            prev_sw = sp_state
                        else:
                            prev_sw = {"qc": qc, "hp": hp, "n_ki": n_ki,
                                       "ets": ets}

                # flush: remaining AVs of the last sweep + trailing norms/Wo
                if "pave" not in prev_sw:
                    prev_sw["pave"] = ps.tile([65, QT_BLK], f32,
                                              name="pav_e", tag="pave",
                                              bufs=1)
                    prev_sw["pavo"] = ps.tile([128, QT_BLK], f32,
                                              name="pav_o", tag="pavo",
                                              bufs=1)
                for idx in range(prev_sw.get("done", 0),
                                 len(prev_sw["ets"])):
                    emit_av(prev_sw, idx)
                last_norm = dens(prev_sw)
                bcast_norm(pend_norm)  # (3, 0)
                # wo(2) split around the final normalize so its matmuls fill
                # the PE while the (3,1) reciprocal chain runs on DVE
                # (dma_split > 1 regressed: column-sliced stores drop the
                # per-partition contiguous run to 256B, halving DMA
                # efficiency - keep whole-tile stores)
                wo(2, jts=range(0, 4))
                bcast_norm(last_norm)  # (3, 1)
                wo(2, jts=range(4, 8), bias_split=True)
                wo(3, bias_split=True)

    nc.compile()
    return nc


def kernel(query, key, value, mask, Wq, bq, Wk, bk, Wv, bv, Wo, bo):
    query = np.asarray(query, np.float32)
    key_ = np.asarray(key, np.float32)
    value = np.asarray(value, np.float32)
    Wq, Wk, Wv, Wo = (np.asarray(w, np.float32) for w in (Wq, Wk, Wv, Wo))
    bq, bk, bv, bo = (np.asarray(b_, np.float32) for b_ in (bq, bk, bv, bo))

    mask = np.asarray(mask)
    assert np.array_equal(mask != 0, np.tril(np.ones((S, S), bool))), \
        "kernel is specialized to the causal mask"
    if "nc" not in _CACHE:
        _CACHE["nc"] = _build()
    nc = _CACHE["nc"]

    def xt(x):  # [S, D] -> [N_QC, 128, KC, QT_BLK] bf16, partition-major
        a = x.T.reshape(KC, 128, S).transpose(1, 0, 2)  # [128, KC, S]
        a = a.reshape(128, KC, N_QC, QT_BLK).transpose(2, 0, 1, 3)
        return np.ascontiguousarray(a).astype(BF16)

    def wslice(W, c):  # [D, D] -> [128, KC, O] bf16 of W[o_slice].T
        hg = c % TP
        a = W[hg * O:(hg + 1) * O].T.reshape(KC, 128, O).transpose(1, 0, 2)
        return np.ascontiguousarray(a).astype(BF16)

    # bcast selector: rows 0-63 of pbc take 1/den_even (rdsb row 64),
    # rows 64-127 take 1/den_odd (rdsb row 0)
    bsel = np.zeros((65, 128), np.float32)
    bsel[64, 0:64] = 1.0
    bsel[0, 64:128] = 1.0

    in_maps = []
    for c in range(N_CORES):
        b_, hg = c // TP, c % TP
        osl = slice(hg * O, (hg + 1) * O)
        bo_part = bo if hg == 0 else np.zeros_like(bo)
        wot = Wo[:, osl].T.reshape(2, 128, D).transpose(1, 0, 2)
        in_maps.append({
            "xqt": xt(query[b_]),
            "xkt": xt(key_[b_]),
            "xvt": xt(value[b_]),
            "wqt": wslice(Wq, c),
            "wkt": wslice(Wk, c),
            "wvt": wslice(Wv, c),
            "wot": np.ascontiguousarray(wot).astype(BF16),
            "bqc": np.ascontiguousarray(bq[osl].reshape(2, 128).T),
            "bkc": np.ascontiguousarray(bk[osl].reshape(2, 128).T),
            "bvb": np.ascontiguousarray(np.broadcast_to(bv[osl], (128, O))),
            "boc": np.ascontiguousarray(bo_part.reshape(8, 128).T),
            "bsel": bsel,
        })

    res = run_bass_kernel_spmd(nc, in_maps, core_ids=list(range(N_CORES)))

    out = np.zeros((B, S, D), np.float32)
    for c in range(N_CORES):
        part = res.results[c]["out"].reshape(D, S)  # out^T [j, s]
        out[c // TP] += part.T.astype(np.float32)
    return out
